# revision 27
# baseline (speedup 1.0000x reference)
"""Trainium2 Bass kernel for nn_BestModel5 (dual-GRU encoder + BxB pair classifier).

Sharding (8 cores): cores 0-3 query-GRU batch shards of 64; cores 4-7 reply-GRU.
Classifier sharded 8-way over the 256 query rows (32 i-rows/core).
Embedding gather + layout prep on host; all matmuls bf16 on PE, f32 PSUM.

GRU step pipeline: x-projections (incl. length-mask row and a ones-row that
carries the gate biases, both zero-padded to K=128 - a short stationary
operand pins LDWEIGHTS to row-group q0 and serializes the PE) are
matmul-accumulated straight into PSUM two steps per bank, so the recurrent
chain is just
  h-MMs -> sigmoid(PSUM) -> r*h -> cand-MMs -> tanh(PSUM) -> zbar*c -> + z*h
with all elementwise ops in bf16 (2x DVE mode) and z*h = h - zbar*h built
off-chain from sigmoid(-zpre).
"""

import numpy as np
import ml_dtypes

BF16 = ml_dtypes.bfloat16

V, E, H, B, T = 100000, 256, 256, 256, 40
D_HID, D_OUT = 256, 2
NCORES = 8
BSH = 64            # batch rows per GRU shard
NSH = 4             # GRU batch shards per encoder
BT = BSH * T        # 2560 columns of xembT per core
IBLK = B // NCORES  # 32 classifier i-rows per core
NPAIR = T // 2      # PSUM step-pairs

_cache = {}


def _build(sim_gelu=False):
    """Build + compile the SPMD Bass program once."""
    import concourse.bacc as bacc
    import concourse.bass as bass
    import concourse.tile as tile
    import concourse.mybir as mybir

    f32 = mybir.dt.float32
    bf16 = mybir.dt.bfloat16
    AF = mybir.ActivationFunctionType

    nc = bacc.Bacc("TRN2", target_bir_lowering=False, debug=False, num_devices=NCORES)

    def din(name, shape, dt):
        return nc.dram_tensor(name, shape, dt, kind="ExternalInput").ap()

    # per-core inputs (content differs per core; shapes identical)
    xembT = din("xembT", [E + 2, BT], bf16)      # rows 0-255 emb, 256 mask, 257 ones
    whg = din("whg", [H, 2 * H], bf16)           # Wg[E:E+H, :]
    wxg = din("wxg", [E, 2 * H], bf16)           # Wg[:E, :]
    wxgm = din("wxgm", [2, 2 * H], bf16)         # row0 = [0|30] mask row, row1 = bg
    wch = din("wch", [H, H], bf16)               # Wc[E:E+H, :]
    wxc = din("wxc", [E, H], bf16)               # Wc[:E, :]
    wxcm = din("wxcm", [2, H], bf16)             # row0 = 0, row1 = bc
    w1q = din("w1q", [H, D_HID], bf16)           # W1[:256]
    w1r = din("w1r", [H, D_HID], bf16)           # W1[257:513]
    wdt = din("wdt", [1, IBLK // 2 * D_HID], bf16)  # W1[256] tiled 16x
    rhsb = din("rhsb", [4, IBLK * B], bf16)      # [0;ones|0;0|0;0;ones] pattern
    b1 = din("b1", [D_HID], f32)
    w2 = din("w2", [D_HID, D_OUT], bf16)

    out = nc.dram_tensor("out", [D_OUT, IBLK * B], f32, kind="ExternalOutput").ap()

    with tile.TileContext(nc) as tc:
        with (
            tc.tile_pool(name="persist", bufs=1) as pp,
            tc.tile_pool(name="dram", bufs=1, space="DRAM") as dramp,
        ):
            # ---- loads: chunk 0 + x-weights first so the recurrence can
            # start immediately; h-weights next; classifier weights deferred ----
            xT = [pp.tile([128, BT], bf16, tag=f"xT{k}", name=f"xT{k}") for k in range(2)]
            xTm = pp.tile([128, BT], bf16, tag="xTm", name="xTm")
            nc.vector.memset(xTm[:, 0:256], 0.0)

            cs0 = slice(0, 256)
            nc.sync.dma_start(xT[0][:, cs0], xembT[0:128, cs0])
            nc.gpsimd.dma_start(xT[1][:, cs0], xembT[128:256, cs0])
            nc.scalar.dma_start(xTm[0:2, cs0], xembT[256:258, cs0])

            wxgm_s = pp.tile([128, 2 * H], bf16, tag="wxgm", name="wxgm")
            nc.vector.memset(wxgm_s[:], 0.0)
            wxcm_s = pp.tile([128, H], bf16, tag="wxcm", name="wxcm")
            nc.vector.memset(wxcm_s[:], 0.0)
            nc.vector.memset(xTm[:, 256:BT], 0.0)

            wxg_s = [pp.tile([128, 2 * H], bf16, tag=f"wxg{k}", name=f"wxg{k}") for k in range(2)]
            nc.sync.dma_start(wxg_s[0][:], wxg[0:128, :])
            nc.gpsimd.dma_start(wxg_s[1][:], wxg[128:256, :])
            nc.scalar.dma_start(wxgm_s[0:2, :], wxgm[:])
            wxc_s = [pp.tile([128, H], bf16, tag=f"wxc{k}", name=f"wxc{k}") for k in range(2)]
            nc.sync.dma_start(wxc_s[0][:], wxc[0:128, :])
            nc.gpsimd.dma_start(wxc_s[1][:], wxc[128:256, :])
            nc.scalar.dma_start(wxcm_s[0:2, :], wxcm[:])

            whg_s = [pp.tile([128, 2 * H], bf16, tag=f"whg{k}", name=f"whg{k}") for k in range(2)]
            nc.sync.dma_start(whg_s[0][:], whg[0:128, :])
            nc.gpsimd.dma_start(whg_s[1][:], whg[128:256, :])
            wch_s = [pp.tile([128, H], bf16, tag=f"wch{k}", name=f"wch{k}") for k in range(2)]
            nc.sync.dma_start(wch_s[0][:], wch[0:128, :])
            nc.gpsimd.dma_start(wch_s[1][:], wch[128:256, :])

            # preload sigmoid/tanh ACT table during DMA wait
            warm_in = pp.tile([1, 2], f32, tag="warmi", name="warmi")
            nc.vector.memset(warm_in[:], 0.0)
            warm_out = pp.tile([1, 2], f32, tag="warmo", name="warmo")
            nc.scalar.activation(warm_out[:], warm_in[:], AF.Sigmoid)

            # remaining xembT in 256-col (2-pair) chunks, in step order
            dq = [nc.sync, nc.gpsimd, nc.scalar]
            for J in range(1, NPAIR // 2):
                cs = slice(256 * J, 256 * J + 256)
                dq[J % 3].dma_start(xT[0][:, cs], xembT[0:128, cs])
                dq[(J + 1) % 3].dma_start(xT[1][:, cs], xembT[128:256, cs])
                dq[(J + 2) % 3].dma_start(xTm[0:2, cs], xembT[256:258, cs])

            # classifier static operands (pad rows 4-127 with zeros so the
            # stationary operand spans all PE row groups)
            lhs4 = pp.tile([128, IBLK // 2 * D_HID], bf16, tag="lhs4", name="lhs4")
            nc.vector.memset(lhs4[:], 0.0)
            rhs4 = pp.tile([128, IBLK * B], bf16, tag="rhs4", name="rhs4")
            nc.vector.memset(rhs4[:], 0.0)

            # ---- GRU recurrence ----
            with (
                tc.tile_pool(name="grpsum", bufs=2, space="PSUM") as grp,
                tc.tile_pool(name="gzpsum", bufs=2, space="PSUM") as gzp,
                tc.tile_pool(name="cpsum", bufs=3, space="PSUM") as cp,
                tc.tile_pool(name="jpsum", bufs=1, space="PSUM") as jp,
                tc.tile_pool(name="step", bufs=4) as sp,
            ):
                def emit_gx(j, pool, mbase, tag):
                    """Gates x-part (2 m-blocks) for pair j; cols 128*m+64*tau+b."""
                    g = pool.tile([128, 512], f32, tag=tag, name=f"{tag}{j}")
                    cs = slice(128 * j, 128 * j + 128)
                    for mi in range(2):
                        m = mbase + mi
                        ms = slice(128 * m, 128 * m + 128)
                        os = slice(128 * mi, 128 * mi + 128)
                        nc.tensor.matmul(g[:, os], wxg_s[0][:, ms], xT[0][:, cs],
                                         start=(mi == 0), stop=False)
                        nc.tensor.matmul(g[:, os], wxg_s[1][:, ms], xT[1][:, cs],
                                         start=False, stop=False)
                        nc.tensor.matmul(g[:, os], wxgm_s[:, ms], xTm[:, cs],
                                         start=False, stop=False)
                    return g

                def emit_cx(j, mi):
                    """Cand x-part m-block mi for pair j (new tile when mi=0)."""
                    if mi == 0:
                        c = cp.tile([128, 512], f32, tag="c", name=f"c{j}")
                        emit_cx.cur = c
                    c = emit_cx.cur
                    cs = slice(128 * j, 128 * j + 128)
                    ms = slice(128 * mi, 128 * mi + 128)
                    nc.tensor.matmul(c[:, ms], wxc_s[0][:, ms], xT[0][:, cs],
                                     start=(mi == 0), stop=False)
                    nc.tensor.matmul(c[:, ms], wxc_s[1][:, ms], xT[1][:, cs],
                                     start=False, stop=False)
                    nc.tensor.matmul(c[:, ms], wxcm_s[:, ms], xTm[:, cs],
                                     start=False, stop=False)
                    return c

                h_bf = pp.tile([128, 128], bf16, tag="hbf", name="hbf", bufs=3)
                nc.vector.memset(h_bf[:], 0.0)

                gr_cur = emit_gx(0, grp, 0, "gr")
                gz_cur = emit_gx(0, gzp, 2, "gz")
                emit_cx(0, 0)
                c_cur = emit_cx(0, 1)
                gr_nxt = gz_nxt = c_nxt = None

                for t in range(T):
                    j, tau = t // 2, t % 2
                    off = 64 * tau

                    # gates h-part: r then z
                    for mi in range(2):
                        for k in range(2):
                            last = (tau == 1 and mi == 1 and k == 1)
                            nc.tensor.matmul(
                                gr_cur[:, 128 * mi + off:128 * mi + off + 64],
                                whg_s[k][:, 128 * mi:128 * mi + 128],
                                h_bf[:, 64 * k:64 * k + 64],
                                start=False, stop=last)
                    for mi in range(2):
                        for k in range(2):
                            last = (tau == 1 and mi == 1 and k == 1)
                            nc.tensor.matmul(
                                gz_cur[:, 128 * mi + off:128 * mi + off + 64],
                                whg_s[k][:, 128 * (mi + 2):128 * (mi + 2) + 128],
                                h_bf[:, 64 * k:64 * k + 64],
                                start=False, stop=last)

                    # x-batch A fills the PE while sigmoid+mul run
                    if j + 1 < NPAIR:
                        if tau == 0:
                            gr_nxt = emit_gx(j + 1, grp, 0, "gr")
                        else:
                            emit_cx(j + 1, 0)

                    gr_v = gr_cur[:, 0:256].rearrange("p (m x) -> p m x", m=2, x=128)
                    gz_v = gz_cur[:, 0:256].rearrange("p (m x) -> p m x", m=2, x=128)
                    r_bf = sp.tile([128, 128], bf16, tag="r", name="r")
                    nc.scalar.activation(
                        r_bf[:].rearrange("p (m b) -> p m b", m=2, b=64),
                        gr_v[:, :, off:off + 64], AF.Sigmoid)
                    zb_bf = sp.tile([128, 128], bf16, tag="zb", name="zb")
                    nc.scalar.activation(
                        zb_bf[:].rearrange("p (m b) -> p m b", m=2, b=64),
                        gz_v[:, :, off:off + 64], AF.Sigmoid, scale=-1.0)

                    rh = sp.tile([128, 128], bf16, tag="rh", name="rh")
                    nc.vector.tensor_mul(rh[:], r_bf[:], h_bf[:])

                    # cand h-part
                    for mi in range(2):
                        for k in range(2):
                            last = (tau == 1 and mi == 1 and k == 1)
                            nc.tensor.matmul(
                                c_cur[:, 128 * mi + off:128 * mi + off + 64],
                                wch_s[k][:, 128 * mi:128 * mi + 128],
                                rh[:, 64 * k:64 * k + 64],
                                start=False, stop=last)

                    # x-batch B in the shadow of tanh + tail
                    if j + 1 < NPAIR:
                        if tau == 0:
                            gz_nxt = emit_gx(j + 1, gzp, 2, "gz")
                        else:
                            c_nxt = emit_cx(j + 1, 1)

                    # z*h = h - zbar*h, off the critical chain
                    s1 = sp.tile([128, 128], bf16, tag="s1", name="s1")
                    nc.vector.tensor_mul(s1[:], zb_bf[:], h_bf[:])
                    hd = sp.tile([128, 128], bf16, tag="hd", name="hd")
                    nc.vector.tensor_sub(hd[:], h_bf[:], s1[:])

                    c_v = c_cur[:, 0:256].rearrange("p (m x) -> p m x", m=2, x=128)
                    c_bf = sp.tile([128, 128], bf16, tag="ct", name="ct")
                    nc.scalar.activation(
                        c_bf[:].rearrange("p (m b) -> p m b", m=2, b=64),
                        c_v[:, :, off:off + 64], AF.Tanh)

                    # pad PE duty to ~100% so the HAM clock-gate can
                    # reach 2.4 GHz through the recurrence
                    for _ in range(2):
                        jnk_g = jp.tile([128, 512], f32, tag="j", name="j")
                        nc.tensor.matmul(jnk_g[:, 0:128], whg_s[0][:, 0:128],
                                         xT[0][:, 0:128], start=True, stop=True)

                    zbc = sp.tile([128, 128], bf16, tag="zbc", name="zbc")
                    nc.vector.tensor_mul(zbc[:], zb_bf[:], c_bf[:])
                    h_new = pp.tile([128, 128], bf16, tag="hbf", name="hbf", bufs=3)
                    nc.vector.tensor_add(h_new[:], zbc[:], hd[:])
                    h_bf = h_new

                    if tau == 1 and j + 1 < NPAIR:
                        gr_cur, gz_cur, c_cur = gr_nxt, gz_nxt, c_nxt

            warm_g = pp.tile([128, 4], f32, tag="warmg", name="warmg")
            nc.vector.tensor_copy(warm_g[:], jnk_g[:, 0:4])

            # ---- exchange encodings ----
            ag_in = dramp.tile([128, 128], bf16, tag="agin", name="agin")
            ag_out = dramp.tile([NCORES, 128, 128], bf16, tag="agout", name="agout")

            nc.sync.dma_start(ag_in[:], h_bf[:])
            nc.gpsimd.collective_compute(
                "AllGather", mybir.AluOpType.bypass,
                replica_groups=[list(range(NCORES))],
                ins=[ag_in.opt()], outs=[ag_out.opt()])

            # classifier weights + gelu ACT table load overlap the collective
            w1q_s = [pp.tile([128, D_HID], bf16, tag=f"w1q{k}", name=f"w1q{k}") for k in range(2)]
            nc.sync.dma_start(w1q_s[0][:], w1q[0:128, :])
            nc.sync.dma_start(w1q_s[1][:], w1q[128:256, :])
            w1r_s = [pp.tile([128, D_HID], bf16, tag=f"w1r{k}", name=f"w1r{k}") for k in range(2)]
            nc.gpsimd.dma_start(w1r_s[0][:], w1r[0:128, :])
            nc.gpsimd.dma_start(w1r_s[1][:], w1r[128:256, :])
            b1_s = pp.tile([128, 2], f32, tag="b1", name="b1")
            nc.scalar.dma_start(b1_s[:], b1.rearrange("(m p) -> p m", p=128))
            w2_s = [pp.tile([128, D_OUT], bf16, tag=f"w2{k}", name=f"w2{k}") for k in range(2)]
            nc.scalar.dma_start(w2_s[0][:], w2[0:128, :])
            nc.scalar.dma_start(w2_s[1][:], w2[128:256, :])
            nc.sync.dma_start(lhs4[0:1, :], wdt[:])
            nc.sync.dma_start(lhs4[2:3, :], wdt[:])
            nc.gpsimd.dma_start(rhs4[0:4, :], rhsb[:])

            gelu_af = mybir.ActivationFunctionType.Tanh if sim_gelu \
                else mybir.ActivationFunctionType.Gelu_apprx_tanh
            warm_out2 = pp.tile([1, 2], f32, tag="warmo2", name="warmo2")
            nc.scalar.activation(warm_out2[:], warm_in[:], gelu_af)

            # per-core q slice: rows [32*core, 32*core+32) live on gathered
            # block core//2, batch-half core%2 -> one register-offset DMA,
            # issued first since the q1->lhs4 staging chain hangs off it
            qloc = pp.tile([128, 2 * IBLK], bf16, tag="qloc", name="qloc")  # [p, c*32+i]
            pid = nc.sync.partition_id()
            agv = ag_out.rearrange("n p (c h b) -> n p c h b", c=2, h=2, b=32)
            nc.sync.dma_start(
                qloc[:].rearrange("p (c b) -> p c b", c=2, b=32),
                agv[bass.ds(pid >> 1, 1), :, :, bass.ds(pid & 1, 1), :])

            # rT2: whole gathered r blocks; col = 128*kshard + 64*chalf + b
            rT2 = pp.tile([128, 512], bf16, tag="rT2", name="rT2")
            qeng = [nc.gpsimd, nc.scalar, nc.sync, nc.gpsimd]
            for k in range(NSH):
                qeng[k].dma_start(rT2[:, 128 * k:128 * k + 128], ag_out[4 + k])
            # per-hidden-half view with j = 64*kshard + b ordering
            rT_v = rT2[:].rearrange("p (k c b) -> p c k b", k=4, c=2, b=64)

            # ---- classifier ----
            with (
                tc.tile_pool(name="spsum", bufs=1, space="PSUM") as sps,
                tc.tile_pool(name="hpsum", bufs=2, space="PSUM") as hps,
                tc.tile_pool(name="lpsum", bufs=1, space="PSUM") as lps,
                tc.tile_pool(name="cls", bufs=3) as cpool,
            ):
                # PE warm-up: the HAM clock-gate leaves the array at 1.2 GHz
                # after the long collective idle; junk matmuls spanning the
                # preamble restore 2.4 GHz before the main classifier stream
                def warm(n):
                    for _ in range(n):
                        jnk = hps.tile([128, 4 * B], f32, tag="hps", name="hps")
                        nc.tensor.matmul(jnk[:, 0:512], whg_s[0][:, 0:128],
                                         rT2[:], start=True, stop=True)
                    return jnk

                # R1T + b1 first: only needs rT2, overlaps the qloc chain
                r1tb = pp.tile([128, 2 * B], f32, tag="r1tb", name="r1tb")
                for m in range(2):
                    ps3 = sps.tile([128, B], f32, tag="sps", name="sps")
                    for k in range(2):
                        nc.tensor.matmul(ps3[:],
                                         w1r_s[k][:, 128 * m:128 * m + 128],
                                         rT_v[:, k],
                                         start=(k == 0), stop=(k == 1))
                    nc.scalar.activation(r1tb[:, 256 * m:256 * m + 256], ps3[:],
                                         AF.Identity, bias=b1_s[:, m:m + 1])

                # Q1 rows for my i's: [32, 256] bf16
                ps = sps.tile([IBLK, D_HID], f32, tag="sps", name="sps")
                for c in range(2):
                    nc.tensor.matmul(ps[:], qloc[:, 32 * c:32 * c + 32],
                                     w1q_s[c][:], start=(c == 0), stop=(c == 1))
                q1 = pp.tile([IBLK, D_HID], bf16, tag="q1", name="q1")
                nc.scalar.activation(q1[:], ps[:], AF.Copy, bias=0.0)
                nc.sync.dma_start(lhs4[1:2, :], q1[0:16, :])
                nc.gpsimd.dma_start(lhs4[3:4, :], q1[16:32, :])

                # dist rows for my i's: [32, 256] bf16
                ps2 = sps.tile([IBLK, B], f32, tag="sps", name="sps")
                for c in range(2):
                    nc.tensor.matmul(ps2[:], qloc[:, 32 * c:32 * c + 32],
                                     rT_v[:, c], start=(c == 0), stop=(c == 1))
                dist = pp.tile([IBLK, B], bf16, tag="dist", name="dist")
                nc.scalar.activation(dist[:], ps2[:], AF.Copy, bias=0.0)
                nc.gpsimd.dma_start(
                    rhs4[0:1, :].rearrange("o (p ii j) -> o p ii j",
                                           p=IBLK // 2, ii=2, j=B)[:, :, 0, :],
                    dist[0:16, :])
                nc.sync.dma_start(
                    rhs4[2:3, :].rearrange("o (p ii j) -> o p ii j",
                                           p=IBLK // 2, ii=2, j=B)[:, :, 1, :],
                    dist[16:32, :])

                r1tb2 = pp.tile([128, 4 * B], f32, tag="r1tb2", name="r1tb2")
                r2v = r1tb2[:].rearrange("p (m ii j) -> p m ii j", m=2, ii=2,
                                         j=B)
                for ii in range(2):
                    nc.vector.tensor_copy(
                        r2v[:, :, ii, :],
                        r1tb[:].rearrange("p (m j) -> p m j", m=2, j=B))

                jnk = warm(12)
                warm_sb = pp.tile([128, 4], f32, tag="warm3", name="warm3")
                nc.vector.tensor_copy(warm_sb[:], jnk[:, 0:4])
                warm_dram = dramp.tile([128, 4], f32, tag="warmd", name="warmd")
                nc.scalar.dma_start(warm_dram[:], warm_sb[:])

                out_sb = pp.tile([D_OUT, IBLK * B], f32, tag="outsb", name="outsb")
                # pair-rows processed two at a time so gelu and the output
                # copy amortize their fixed cost over 1024 columns; the output
                # copy alternates Scalar/Vector to balance engine load (b2 is
                # added on the host)
                # r1tb4: r1 operand duplicated over (m, sub) for 1024-wide adds
                r1tb4 = pp.tile([128, 8 * B], f32, tag="r1tb4", name="r1tb4")
                for m in range(2):
                    for sub in range(2):
                        nc.vector.tensor_copy(
                            r1tb4[:, 1024 * m + 512 * sub:
                                  1024 * m + 512 * sub + 512],
                            r1tb2[:, 512 * m:512 * m + 512])

                def emit_w2(prp, h1):
                    """Second layer + output copy for pair-row-pair prp."""
                    l_ps = lps.tile([D_OUT, 4 * B], f32, tag="lps", name="lps")
                    for sub in range(2):
                        for k in range(2):
                            nc.tensor.matmul(
                                l_ps[:, 512 * sub:512 * sub + 512], w2_s[k][:],
                                h1[:, 1024 * k + 512 * sub:
                                   1024 * k + 512 * sub + 512],
                                start=(k == 0), stop=(k == 1))
                    oseg = out_sb[:, 1024 * prp:1024 * prp + 1024]
                    if prp % 2 == 0:
                        nc.scalar.activation(oseg, l_ps[:], AF.Copy, bias=0.0)
                    else:
                        nc.vector.tensor_copy(oseg, l_ps[:])
                    [nc.sync, nc.gpsimd][prp % 2].dma_start(
                        out[:, 1024 * prp:1024 * prp + 1024], oseg)

                # software-pipelined: W2(prp-1) is emitted after the h1 MMs of
                # prp so the in-order PE never stalls on gelu(prp)
                prev = None
                for prp in range(IBLK // 4):
                    pr0 = 2 * prp
                    # h1 block layout: col = 1024*m + 512*sub + 256*ii + j
                    h1p4 = cpool.tile([128, 8 * B], bf16, tag="h1p", name="h1p")
                    for m in range(2):
                        h_ps = hps.tile([128, 4 * B], f32, tag="hps",
                                        name="hps")
                        for sub in range(2):
                            pr = pr0 + sub
                            nc.tensor.matmul(
                                h_ps[:, 512 * sub:512 * sub + 512],
                                lhs4[0:128, D_HID * pr + 128 * m:
                                     D_HID * pr + 128 * m + 128],
                                rhs4[0:128, 2 * B * pr:2 * B * pr + 2 * B],
                                start=True, stop=True)
                        nc.vector.tensor_add(
                            h1p4[:, 1024 * m:1024 * m + 1024], h_ps[:],
                            r1tb4[:, 1024 * m:1024 * m + 1024])
                    h1 = cpool.tile([128, 8 * B], bf16, tag="h1", name="h1")
                    nc.scalar.activation(h1[:], h1p4[:], gelu_af)
                    if prev is not None:
                        emit_w2(prev[0], prev[1])
                    prev = (prp, h1)
                emit_w2(prev[0], prev[1])

    nc.compile()
    return nc


def _rhs_base():
    """[4, IBLK*B] pattern: per 512-col pair-block rows are
    [0,0],[ones,0],[0,0],[0,ones] - dist blocks get DMA'd in on device."""
    r = np.zeros((4, IBLK * B), dtype=BF16)
    v = r.reshape(4, IBLK // 2, 2, B)
    v[1, :, 0, :] = 1.0
    v[3, :, 1, :] = 1.0
    return r


def _prep_inputs(inputs):
    """Host-side prep: embed+transpose sequences, split weights, per-core maps."""
    emb = inputs["embeddings"]
    in_maps = []
    f32 = np.float32

    # classifier tensors (identical on all cores)
    W1, b1, W2, b2 = (inputs["W1"], inputs["b1"], inputs["W2"], inputs["b2"])
    common = {
        "w1q": np.ascontiguousarray(W1[:H]).astype(BF16),
        "w1r": np.ascontiguousarray(W1[H + 1:]).astype(BF16),
        "wdt": np.tile(np.ascontiguousarray(W1[H:H + 1]).astype(BF16),
                       (1, IBLK // 2)),
        "rhsb": _rhs_base(),
        "b1": b1.astype(f32),
        "w2": W2.astype(BF16),
    }

    ones_row = np.ones((1, BT), f32)
    for core in range(NCORES):
        enc = core // NSH
        s = core % NSH
        if enc == 0:
            seqs, lens = inputs["input_queries"], inputs["query_lengths"]
            Wg, bgv, Wc, bcv = (inputs["Wg_q"], inputs["bg_q"],
                                inputs["Wc_q"], inputs["bc_q"])
        else:
            seqs, lens = inputs["input_replies"], inputs["reply_lengths"]
            Wg, bgv, Wc, bcv = (inputs["Wg_r"], inputs["bg_r"],
                                inputs["Wc_r"], inputs["bc_r"])
        rows = slice(BSH * s, BSH * s + BSH)
        xe = emb[seqs[rows]]                       # [64, 40, 256]
        xT = np.transpose(xe, (2, 1, 0)).reshape(E, BT)  # col = t*64+b
        lmask = (np.arange(T)[:, None] >= lens[rows][None, :]) \
            .astype(f32).reshape(1, BT)
        xembT = np.concatenate([xT, lmask, ones_row], axis=0).astype(BF16)

        mask_row = np.concatenate([np.zeros(H, f32), np.full(H, 30.0, f32)])
        wxgm = np.stack([mask_row, bgv.astype(f32)]).astype(BF16)
        wxcm = np.stack([np.zeros(H, f32), bcv.astype(f32)]).astype(BF16)

        m = {
            "xembT": xembT,
            "whg": np.ascontiguousarray(Wg[E:]).astype(BF16),
            "wxg": np.ascontiguousarray(Wg[:E]).astype(BF16),
            "wxgm": wxgm,
            "wch": np.ascontiguousarray(Wc[E:]).astype(BF16),
            "wxc": np.ascontiguousarray(Wc[:E]).astype(BF16),
            "wxcm": wxcm,
        }
        m.update(common)
        in_maps.append(m)
    return in_maps


def run_cores(in_maps, trace=False):
    from concourse.bass_utils import run_bass_kernel_spmd
    from concourse.bass_interp import get_hw_module

    if "nc" not in _cache:
        _cache["nc"] = _build()
    nc = _cache["nc"]
    old = nc.m
    nc.m = _cache.setdefault("hwm", get_hw_module(nc.m))
    try:
        res = run_bass_kernel_spmd(nc, in_maps, core_ids=list(range(NCORES)),
                                   trace=trace)
    finally:
        nc.m = old
    return res


def kernel(**inputs):
    in_maps = _prep_inputs(inputs)
    res = run_cores(in_maps)
    logits = np.zeros((B, B, 2), np.float32)
    for core in range(NCORES):
        o = res.results[core]["out"]               # [2, 32*256]
        # pair layout: col = 512*pr + 256*ii + j, local row = 16*ii + pr
        logits[IBLK * core:IBLK * core + IBLK] = \
            o.reshape(2, 16, 2, B).transpose(2, 1, 3, 0).reshape(IBLK, B, 2)
    logits += inputs["b2"].astype(np.float32)[None, None, :]
    pos = logits[np.arange(B), np.arange(B)]
    qi, ri = np.nonzero(~np.eye(B, dtype=bool))
    neg = logits[qi, ri]
    return np.concatenate([pos, neg], axis=0).astype(np.float32)


if __name__ == "__main__":
    _build()
    print("build OK")


# revision 28
# speedup vs baseline: 1.2244x; 1.2244x over previous
"""Trainium2 Bass kernel for nn_BestModel5 (dual-GRU encoder + BxB pair classifier).

Sharding (8 cores): cores 0-3 query-GRU batch shards of 64; cores 4-7 reply-GRU.
Classifier sharded 8-way over the 256 query rows (32 i-rows/core).
Embedding gather + layout prep on host; all matmuls bf16 on PE, f32 PSUM.

GRU step pipeline: x-projections (incl. length-mask row and a ones-row that
carries the gate biases, both zero-padded to K=128 - a short stationary
operand pins LDWEIGHTS to row-group q0 and serializes the PE) are
matmul-accumulated straight into PSUM two steps per bank, so the recurrent
chain is just
  h-MMs -> sigmoid(PSUM) -> r*h -> cand-MMs -> tanh(PSUM) -> zbar*c -> + z*h
with all elementwise ops in bf16 (2x DVE mode) and z*h = h - zbar*h built
off-chain from sigmoid(-zpre).
"""

import numpy as np
import ml_dtypes

BF16 = ml_dtypes.bfloat16

V, E, H, B, T = 100000, 256, 256, 256, 40
D_HID, D_OUT = 256, 2
NCORES = 8
BSH = 64            # batch rows per GRU shard
NSH = 4             # GRU batch shards per encoder
BT = BSH * T        # 2560 columns of xembT per core
IBLK = B // NCORES  # 32 classifier i-rows per core
NPAIR = T // 2      # PSUM step-pairs

_cache = {}


def _build(sim_gelu=False):
    """Build + compile the SPMD Bass program once."""
    import concourse.bacc as bacc
    import concourse.bass as bass
    import concourse.tile as tile
    import concourse.mybir as mybir

    f32 = mybir.dt.float32
    bf16 = mybir.dt.bfloat16
    AF = mybir.ActivationFunctionType

    nc = bacc.Bacc("TRN2", target_bir_lowering=False, debug=False, num_devices=NCORES)

    def din(name, shape, dt):
        return nc.dram_tensor(name, shape, dt, kind="ExternalInput").ap()

    # per-core inputs (content differs per core; shapes identical)
    xembT = din("xembT", [E + 2, BT], bf16)      # rows 0-255 emb, 256 mask, 257 ones
    whg = din("whg", [H, 2 * H], bf16)           # Wg[E:E+H, :]
    wxg = din("wxg", [E, 2 * H], bf16)           # Wg[:E, :]
    wxgm = din("wxgm", [2, 2 * H], bf16)         # row0 = [0|30] mask row, row1 = bg
    wch = din("wch", [H, H], bf16)               # Wc[E:E+H, :]
    wxc = din("wxc", [E, H], bf16)               # Wc[:E, :]
    wxcm = din("wxcm", [2, H], bf16)             # row0 = 0, row1 = bc
    w1q = din("w1q", [H, D_HID], bf16)           # W1[:256]
    w1r = din("w1r", [H, D_HID], bf16)           # W1[257:513]
    wdt = din("wdt", [1, IBLK // 2 * D_HID], bf16)  # W1[256] tiled 16x
    rhsb = din("rhsb", [4, IBLK * B], bf16)      # [0;ones|0;0|0;0;ones] pattern
    b1 = din("b1", [D_HID], f32)
    w2 = din("w2", [D_HID, D_OUT], bf16)

    out = nc.dram_tensor("out", [D_OUT, IBLK * B], f32, kind="ExternalOutput").ap()

    with tile.TileContext(nc) as tc:
        with (
            tc.tile_pool(name="persist", bufs=1) as pp,
            tc.tile_pool(name="dram", bufs=1, space="DRAM") as dramp,
        ):
            # ---- loads: chunk 0 + x-weights first so the recurrence can
            # start immediately; h-weights next; classifier weights deferred ----
            xT = [pp.tile([128, BT], bf16, tag=f"xT{k}", name=f"xT{k}") for k in range(2)]
            xTm = pp.tile([128, BT], bf16, tag="xTm", name="xTm")
            nc.vector.memset(xTm[:, 0:256], 0.0)

            cs0 = slice(0, 256)
            nc.sync.dma_start(xT[0][:, cs0], xembT[0:128, cs0])
            nc.gpsimd.dma_start(xT[1][:, cs0], xembT[128:256, cs0])
            nc.scalar.dma_start(xTm[0:2, cs0], xembT[256:258, cs0])

            wxgm_s = pp.tile([128, 2 * H], bf16, tag="wxgm", name="wxgm")
            nc.vector.memset(wxgm_s[:], 0.0)
            wxcm_s = pp.tile([128, H], bf16, tag="wxcm", name="wxcm")
            nc.vector.memset(wxcm_s[:], 0.0)
            nc.vector.memset(xTm[:, 256:BT], 0.0)

            wxg_s = [pp.tile([128, 2 * H], bf16, tag=f"wxg{k}", name=f"wxg{k}") for k in range(2)]
            nc.sync.dma_start(wxg_s[0][:], wxg[0:128, :])
            nc.gpsimd.dma_start(wxg_s[1][:], wxg[128:256, :])
            nc.scalar.dma_start(wxgm_s[0:2, :], wxgm[:])
            wxc_s = [pp.tile([128, H], bf16, tag=f"wxc{k}", name=f"wxc{k}") for k in range(2)]
            nc.sync.dma_start(wxc_s[0][:], wxc[0:128, :])
            nc.gpsimd.dma_start(wxc_s[1][:], wxc[128:256, :])
            nc.scalar.dma_start(wxcm_s[0:2, :], wxcm[:])

            whg_s = [pp.tile([128, 2 * H], bf16, tag=f"whg{k}", name=f"whg{k}") for k in range(2)]
            nc.sync.dma_start(whg_s[0][:], whg[0:128, :])
            nc.gpsimd.dma_start(whg_s[1][:], whg[128:256, :])
            wch_s = [pp.tile([128, H], bf16, tag=f"wch{k}", name=f"wch{k}") for k in range(2)]
            nc.sync.dma_start(wch_s[0][:], wch[0:128, :])
            nc.gpsimd.dma_start(wch_s[1][:], wch[128:256, :])

            # preload sigmoid/tanh ACT table during DMA wait
            warm_in = pp.tile([1, 2], f32, tag="warmi", name="warmi")
            nc.vector.memset(warm_in[:], 0.0)
            warm_out = pp.tile([1, 2], f32, tag="warmo", name="warmo")
            nc.scalar.activation(warm_out[:], warm_in[:], AF.Sigmoid)

            # remaining xembT in 256-col (2-pair) chunks, in step order
            dq = [nc.sync, nc.gpsimd, nc.scalar]
            for J in range(1, NPAIR // 2):
                cs = slice(256 * J, 256 * J + 256)
                dq[J % 3].dma_start(xT[0][:, cs], xembT[0:128, cs])
                dq[(J + 1) % 3].dma_start(xT[1][:, cs], xembT[128:256, cs])
                dq[(J + 2) % 3].dma_start(xTm[0:2, cs], xembT[256:258, cs])

            # classifier static operands (pad rows 4-127 with zeros so the
            # stationary operand spans all PE row groups)
            lhs4 = pp.tile([128, IBLK // 2 * D_HID], bf16, tag="lhs4", name="lhs4")
            nc.vector.memset(lhs4[:], 0.0)
            rhs4 = pp.tile([128, IBLK * B], bf16, tag="rhs4", name="rhs4")
            nc.vector.memset(rhs4[:], 0.0)

            # ---- GRU recurrence ----
            with (
                tc.tile_pool(name="grpsum", bufs=2, space="PSUM") as grp,
                tc.tile_pool(name="gzpsum", bufs=2, space="PSUM") as gzp,
                tc.tile_pool(name="cpsum", bufs=3, space="PSUM") as cp,
                tc.tile_pool(name="step", bufs=4) as sp,
            ):
                def emit_gx(j, pool, mbase, tag):
                    """Gates x-part (2 m-blocks) for pair j; cols 128*m+64*tau+b."""
                    g = pool.tile([128, 512], f32, tag=tag, name=f"{tag}{j}")
                    cs = slice(128 * j, 128 * j + 128)
                    for mi in range(2):
                        m = mbase + mi
                        ms = slice(128 * m, 128 * m + 128)
                        os = slice(128 * mi, 128 * mi + 128)
                        nc.tensor.matmul(g[:, os], wxg_s[0][:, ms], xT[0][:, cs],
                                         start=(mi == 0), stop=False)
                        nc.tensor.matmul(g[:, os], wxg_s[1][:, ms], xT[1][:, cs],
                                         start=False, stop=False)
                        nc.tensor.matmul(g[:, os], wxgm_s[:, ms], xTm[:, cs],
                                         start=False, stop=False)
                    return g

                def emit_cx(j, mi):
                    """Cand x-part m-block mi for pair j (new tile when mi=0)."""
                    if mi == 0:
                        c = cp.tile([128, 512], f32, tag="c", name=f"c{j}")
                        emit_cx.cur = c
                    c = emit_cx.cur
                    cs = slice(128 * j, 128 * j + 128)
                    ms = slice(128 * mi, 128 * mi + 128)
                    nc.tensor.matmul(c[:, ms], wxc_s[0][:, ms], xT[0][:, cs],
                                     start=(mi == 0), stop=False)
                    nc.tensor.matmul(c[:, ms], wxc_s[1][:, ms], xT[1][:, cs],
                                     start=False, stop=False)
                    nc.tensor.matmul(c[:, ms], wxcm_s[:, ms], xTm[:, cs],
                                     start=False, stop=False)
                    return c

                h_bf = pp.tile([128, 128], bf16, tag="hbf", name="hbf", bufs=3)
                nc.vector.memset(h_bf[:], 0.0)

                gr_cur = emit_gx(0, grp, 0, "gr")
                gz_cur = emit_gx(0, gzp, 2, "gz")
                emit_cx(0, 0)
                c_cur = emit_cx(0, 1)
                gr_nxt = gz_nxt = c_nxt = None

                for t in range(T):
                    j, tau = t // 2, t % 2
                    off = 64 * tau

                    # gates h-part: r then z
                    for mi in range(2):
                        for k in range(2):
                            last = (tau == 1 and mi == 1 and k == 1)
                            nc.tensor.matmul(
                                gr_cur[:, 128 * mi + off:128 * mi + off + 64],
                                whg_s[k][:, 128 * mi:128 * mi + 128],
                                h_bf[:, 64 * k:64 * k + 64],
                                start=False, stop=last)
                    for mi in range(2):
                        for k in range(2):
                            last = (tau == 1 and mi == 1 and k == 1)
                            nc.tensor.matmul(
                                gz_cur[:, 128 * mi + off:128 * mi + off + 64],
                                whg_s[k][:, 128 * (mi + 2):128 * (mi + 2) + 128],
                                h_bf[:, 64 * k:64 * k + 64],
                                start=False, stop=last)

                    # x-batch A fills the PE while sigmoid+mul run
                    if j + 1 < NPAIR:
                        if tau == 0:
                            gr_nxt = emit_gx(j + 1, grp, 0, "gr")
                        else:
                            emit_cx(j + 1, 0)

                    gr_v = gr_cur[:, 0:256].rearrange("p (m x) -> p m x", m=2, x=128)
                    gz_v = gz_cur[:, 0:256].rearrange("p (m x) -> p m x", m=2, x=128)
                    r_bf = sp.tile([128, 128], bf16, tag="r", name="r")
                    nc.scalar.activation(
                        r_bf[:].rearrange("p (m b) -> p m b", m=2, b=64),
                        gr_v[:, :, off:off + 64], AF.Sigmoid)
                    zb_bf = sp.tile([128, 128], bf16, tag="zb", name="zb")
                    nc.scalar.activation(
                        zb_bf[:].rearrange("p (m b) -> p m b", m=2, b=64),
                        gz_v[:, :, off:off + 64], AF.Sigmoid, scale=-1.0)

                    rh = sp.tile([128, 128], bf16, tag="rh", name="rh")
                    nc.vector.tensor_mul(rh[:], r_bf[:], h_bf[:])

                    # cand h-part
                    for mi in range(2):
                        for k in range(2):
                            last = (tau == 1 and mi == 1 and k == 1)
                            nc.tensor.matmul(
                                c_cur[:, 128 * mi + off:128 * mi + off + 64],
                                wch_s[k][:, 128 * mi:128 * mi + 128],
                                rh[:, 64 * k:64 * k + 64],
                                start=False, stop=last)

                    # x-batch B in the shadow of tanh + tail
                    if j + 1 < NPAIR:
                        if tau == 0:
                            gz_nxt = emit_gx(j + 1, gzp, 2, "gz")
                        else:
                            c_nxt = emit_cx(j + 1, 1)

                    # z*h = h - zbar*h, off the critical chain
                    s1 = sp.tile([128, 128], bf16, tag="s1", name="s1")
                    nc.vector.tensor_mul(s1[:], zb_bf[:], h_bf[:])
                    hd = sp.tile([128, 128], bf16, tag="hd", name="hd")
                    nc.vector.tensor_sub(hd[:], h_bf[:], s1[:])

                    c_v = c_cur[:, 0:256].rearrange("p (m x) -> p m x", m=2, x=128)
                    c_bf = sp.tile([128, 128], bf16, tag="ct", name="ct")
                    nc.scalar.activation(
                        c_bf[:].rearrange("p (m b) -> p m b", m=2, b=64),
                        c_v[:, :, off:off + 64], AF.Tanh)

                    zbc = sp.tile([128, 128], bf16, tag="zbc", name="zbc")
                    nc.vector.tensor_mul(zbc[:], zb_bf[:], c_bf[:])
                    h_new = pp.tile([128, 128], bf16, tag="hbf", name="hbf", bufs=3)
                    nc.vector.tensor_add(h_new[:], zbc[:], hd[:])
                    h_bf = h_new

                    if tau == 1 and j + 1 < NPAIR:
                        gr_cur, gz_cur, c_cur = gr_nxt, gz_nxt, c_nxt

            # ---- exchange encodings ----
            ag_in = dramp.tile([128, 128], bf16, tag="agin", name="agin")
            ag_out = dramp.tile([NCORES, 128, 128], bf16, tag="agout", name="agout")

            nc.sync.dma_start(ag_in[:], h_bf[:])
            nc.gpsimd.collective_compute(
                "AllGather", mybir.AluOpType.bypass,
                replica_groups=[list(range(NCORES))],
                ins=[ag_in.opt()], outs=[ag_out.opt()])

            # classifier weights + gelu ACT table load overlap the collective
            w1q_s = [pp.tile([128, D_HID], bf16, tag=f"w1q{k}", name=f"w1q{k}") for k in range(2)]
            nc.sync.dma_start(w1q_s[0][:], w1q[0:128, :])
            nc.sync.dma_start(w1q_s[1][:], w1q[128:256, :])
            w1r_s = [pp.tile([128, D_HID], bf16, tag=f"w1r{k}", name=f"w1r{k}") for k in range(2)]
            nc.gpsimd.dma_start(w1r_s[0][:], w1r[0:128, :])
            nc.gpsimd.dma_start(w1r_s[1][:], w1r[128:256, :])
            b1_s = pp.tile([128, 2], f32, tag="b1", name="b1")
            nc.scalar.dma_start(b1_s[:], b1.rearrange("(m p) -> p m", p=128))
            w2_s = [pp.tile([128, D_OUT], bf16, tag=f"w2{k}", name=f"w2{k}") for k in range(2)]
            nc.scalar.dma_start(w2_s[0][:], w2[0:128, :])
            nc.scalar.dma_start(w2_s[1][:], w2[128:256, :])
            nc.sync.dma_start(lhs4[0:1, :], wdt[:])
            nc.sync.dma_start(lhs4[2:3, :], wdt[:])
            nc.gpsimd.dma_start(rhs4[0:4, :], rhsb[:])

            gelu_af = mybir.ActivationFunctionType.Tanh if sim_gelu \
                else mybir.ActivationFunctionType.Gelu_apprx_tanh
            warm_out2 = pp.tile([1, 2], f32, tag="warmo2", name="warmo2")
            nc.scalar.activation(warm_out2[:], warm_in[:], gelu_af)

            # per-core q slice: rows [32*core, 32*core+32) live on gathered
            # block core//2, batch-half core%2 -> one register-offset DMA,
            # issued first since the q1->lhs4 staging chain hangs off it
            qloc = pp.tile([128, 2 * IBLK], bf16, tag="qloc", name="qloc")  # [p, c*32+i]
            pid = nc.sync.partition_id()
            agv = ag_out.rearrange("n p (c h b) -> n p c h b", c=2, h=2, b=32)
            nc.sync.dma_start(
                qloc[:].rearrange("p (c b) -> p c b", c=2, b=32),
                agv[bass.ds(pid >> 1, 1), :, :, bass.ds(pid & 1, 1), :])

            # rT2: whole gathered r blocks; col = 128*kshard + 64*chalf + b
            rT2 = pp.tile([128, 512], bf16, tag="rT2", name="rT2")
            qeng = [nc.gpsimd, nc.scalar, nc.sync, nc.gpsimd]
            for k in range(NSH):
                qeng[k].dma_start(rT2[:, 128 * k:128 * k + 128], ag_out[4 + k])
            # per-hidden-half view with j = 64*kshard + b ordering
            rT_v = rT2[:].rearrange("p (k c b) -> p c k b", k=4, c=2, b=64)

            # ---- classifier ----
            with (
                tc.tile_pool(name="spsum", bufs=1, space="PSUM") as sps,
                tc.tile_pool(name="hpsum", bufs=2, space="PSUM") as hps,
                tc.tile_pool(name="lpsum", bufs=1, space="PSUM") as lps,
                tc.tile_pool(name="cls", bufs=3) as cpool,
            ):
                # PE warm-up: the HAM clock-gate leaves the array at 1.2 GHz
                # after the long collective idle; junk matmuls spanning the
                # preamble restore 2.4 GHz before the main classifier stream
                def warm(n):
                    for _ in range(n):
                        jnk = hps.tile([128, 4 * B], f32, tag="hps", name="hps")
                        nc.tensor.matmul(jnk[:, 0:512], whg_s[0][:, 0:128],
                                         rT2[:], start=True, stop=True)
                    return jnk

                # R1T + b1 first: only needs rT2, overlaps the qloc chain
                r1tb = pp.tile([128, 2 * B], f32, tag="r1tb", name="r1tb")
                for m in range(2):
                    ps3 = sps.tile([128, B], f32, tag="sps", name="sps")
                    for k in range(2):
                        nc.tensor.matmul(ps3[:],
                                         w1r_s[k][:, 128 * m:128 * m + 128],
                                         rT_v[:, k],
                                         start=(k == 0), stop=(k == 1))
                    nc.scalar.activation(r1tb[:, 256 * m:256 * m + 256], ps3[:],
                                         AF.Identity, bias=b1_s[:, m:m + 1])

                # Q1 rows for my i's: [32, 256] bf16
                ps = sps.tile([IBLK, D_HID], f32, tag="sps", name="sps")
                for c in range(2):
                    nc.tensor.matmul(ps[:], qloc[:, 32 * c:32 * c + 32],
                                     w1q_s[c][:], start=(c == 0), stop=(c == 1))
                q1 = pp.tile([IBLK, D_HID], bf16, tag="q1", name="q1")
                nc.scalar.activation(q1[:], ps[:], AF.Copy, bias=0.0)
                nc.sync.dma_start(lhs4[1:2, :], q1[0:16, :])
                nc.gpsimd.dma_start(lhs4[3:4, :], q1[16:32, :])

                # dist rows for my i's: [32, 256] bf16
                ps2 = sps.tile([IBLK, B], f32, tag="sps", name="sps")
                for c in range(2):
                    nc.tensor.matmul(ps2[:], qloc[:, 32 * c:32 * c + 32],
                                     rT_v[:, c], start=(c == 0), stop=(c == 1))
                dist = pp.tile([IBLK, B], bf16, tag="dist", name="dist")
                nc.scalar.activation(dist[:], ps2[:], AF.Copy, bias=0.0)
                nc.gpsimd.dma_start(
                    rhs4[0:1, :].rearrange("o (p ii j) -> o p ii j",
                                           p=IBLK // 2, ii=2, j=B)[:, :, 0, :],
                    dist[0:16, :])
                nc.sync.dma_start(
                    rhs4[2:3, :].rearrange("o (p ii j) -> o p ii j",
                                           p=IBLK // 2, ii=2, j=B)[:, :, 1, :],
                    dist[16:32, :])

                r1tb2 = pp.tile([128, 4 * B], f32, tag="r1tb2", name="r1tb2")
                r2v = r1tb2[:].rearrange("p (m ii j) -> p m ii j", m=2, ii=2,
                                         j=B)
                for ii in range(2):
                    nc.vector.tensor_copy(
                        r2v[:, :, ii, :],
                        r1tb[:].rearrange("p (m j) -> p m j", m=2, j=B))

                jnk = warm(12)
                warm_sb = pp.tile([128, 4], f32, tag="warm3", name="warm3")
                nc.vector.tensor_copy(warm_sb[:], jnk[:, 0:4])
                warm_dram = dramp.tile([128, 4], f32, tag="warmd", name="warmd")
                nc.scalar.dma_start(warm_dram[:], warm_sb[:])

                out_sb = pp.tile([D_OUT, IBLK * B], f32, tag="outsb", name="outsb")
                # pair-rows processed two at a time so gelu and the output
                # copy amortize their fixed cost over 1024 columns; the output
                # copy alternates Scalar/Vector to balance engine load (b2 is
                # added on the host)
                # r1tb4: r1 operand duplicated over (m, sub) for 1024-wide adds
                r1tb4 = pp.tile([128, 8 * B], f32, tag="r1tb4", name="r1tb4")
                for m in range(2):
                    for sub in range(2):
                        nc.vector.tensor_copy(
                            r1tb4[:, 1024 * m + 512 * sub:
                                  1024 * m + 512 * sub + 512],
                            r1tb2[:, 512 * m:512 * m + 512])

                def emit_w2(prp, h1):
                    """Second layer + output copy for pair-row-pair prp."""
                    l_ps = lps.tile([D_OUT, 4 * B], f32, tag="lps", name="lps")
                    for sub in range(2):
                        for k in range(2):
                            nc.tensor.matmul(
                                l_ps[:, 512 * sub:512 * sub + 512], w2_s[k][:],
                                h1[:, 1024 * k + 512 * sub:
                                   1024 * k + 512 * sub + 512],
                                start=(k == 0), stop=(k == 1))
                    oseg = out_sb[:, 1024 * prp:1024 * prp + 1024]
                    if prp % 2 == 0:
                        nc.scalar.activation(oseg, l_ps[:], AF.Copy, bias=0.0)
                    else:
                        nc.vector.tensor_copy(oseg, l_ps[:])
                    [nc.sync, nc.gpsimd][prp % 2].dma_start(
                        out[:, 1024 * prp:1024 * prp + 1024], oseg)

                # software-pipelined: W2(prp-1) is emitted after the h1 MMs of
                # prp so the in-order PE never stalls on gelu(prp)
                prev = None
                for prp in range(IBLK // 4):
                    pr0 = 2 * prp
                    # h1 block layout: col = 1024*m + 512*sub + 256*ii + j
                    h1p4 = cpool.tile([128, 8 * B], bf16, tag="h1p", name="h1p")
                    for m in range(2):
                        h_ps = hps.tile([128, 4 * B], f32, tag="hps",
                                        name="hps")
                        for sub in range(2):
                            pr = pr0 + sub
                            nc.tensor.matmul(
                                h_ps[:, 512 * sub:512 * sub + 512],
                                lhs4[0:128, D_HID * pr + 128 * m:
                                     D_HID * pr + 128 * m + 128],
                                rhs4[0:128, 2 * B * pr:2 * B * pr + 2 * B],
                                start=True, stop=True)
                        nc.vector.tensor_add(
                            h1p4[:, 1024 * m:1024 * m + 1024], h_ps[:],
                            r1tb4[:, 1024 * m:1024 * m + 1024])
                    h1 = cpool.tile([128, 8 * B], bf16, tag="h1", name="h1")
                    nc.scalar.activation(h1[:], h1p4[:], gelu_af)
                    if prev is not None:
                        emit_w2(prev[0], prev[1])
                    prev = (prp, h1)
                emit_w2(prev[0], prev[1])

    nc.compile()
    return nc


def _rhs_base():
    """[4, IBLK*B] pattern: per 512-col pair-block rows are
    [0,0],[ones,0],[0,0],[0,ones] - dist blocks get DMA'd in on device."""
    r = np.zeros((4, IBLK * B), dtype=BF16)
    v = r.reshape(4, IBLK // 2, 2, B)
    v[1, :, 0, :] = 1.0
    v[3, :, 1, :] = 1.0
    return r


def _prep_inputs(inputs):
    """Host-side prep: embed+transpose sequences, split weights, per-core maps."""
    emb = inputs["embeddings"]
    in_maps = []
    f32 = np.float32

    # classifier tensors (identical on all cores)
    W1, b1, W2, b2 = (inputs["W1"], inputs["b1"], inputs["W2"], inputs["b2"])
    common = {
        "w1q": np.ascontiguousarray(W1[:H]).astype(BF16),
        "w1r": np.ascontiguousarray(W1[H + 1:]).astype(BF16),
        "wdt": np.tile(np.ascontiguousarray(W1[H:H + 1]).astype(BF16),
                       (1, IBLK // 2)),
        "rhsb": _rhs_base(),
        "b1": b1.astype(f32),
        "w2": W2.astype(BF16),
    }

    ones_row = np.ones((1, BT), f32)
    for core in range(NCORES):
        enc = core // NSH
        s = core % NSH
        if enc == 0:
            seqs, lens = inputs["input_queries"], inputs["query_lengths"]
            Wg, bgv, Wc, bcv = (inputs["Wg_q"], inputs["bg_q"],
                                inputs["Wc_q"], inputs["bc_q"])
        else:
            seqs, lens = inputs["input_replies"], inputs["reply_lengths"]
            Wg, bgv, Wc, bcv = (inputs["Wg_r"], inputs["bg_r"],
                                inputs["Wc_r"], inputs["bc_r"])
        rows = slice(BSH * s, BSH * s + BSH)
        xe = emb[seqs[rows]]                       # [64, 40, 256]
        xT = np.transpose(xe, (2, 1, 0)).reshape(E, BT)  # col = t*64+b
        lmask = (np.arange(T)[:, None] >= lens[rows][None, :]) \
            .astype(f32).reshape(1, BT)
        xembT = np.concatenate([xT, lmask, ones_row], axis=0).astype(BF16)

        mask_row = np.concatenate([np.zeros(H, f32), np.full(H, 30.0, f32)])
        wxgm = np.stack([mask_row, bgv.astype(f32)]).astype(BF16)
        wxcm = np.stack([np.zeros(H, f32), bcv.astype(f32)]).astype(BF16)

        m = {
            "xembT": xembT,
            "whg": np.ascontiguousarray(Wg[E:]).astype(BF16),
            "wxg": np.ascontiguousarray(Wg[:E]).astype(BF16),
            "wxgm": wxgm,
            "wch": np.ascontiguousarray(Wc[E:]).astype(BF16),
            "wxc": np.ascontiguousarray(Wc[:E]).astype(BF16),
            "wxcm": wxcm,
        }
        m.update(common)
        in_maps.append(m)
    return in_maps


def run_cores(in_maps, trace=False):
    from concourse.bass_utils import run_bass_kernel_spmd
    from concourse.bass_interp import get_hw_module

    if "nc" not in _cache:
        _cache["nc"] = _build()
    nc = _cache["nc"]
    old = nc.m
    nc.m = _cache.setdefault("hwm", get_hw_module(nc.m))
    try:
        res = run_bass_kernel_spmd(nc, in_maps, core_ids=list(range(NCORES)),
                                   trace=trace)
    finally:
        nc.m = old
    return res


def kernel(**inputs):
    in_maps = _prep_inputs(inputs)
    res = run_cores(in_maps)
    logits = np.zeros((B, B, 2), np.float32)
    for core in range(NCORES):
        o = res.results[core]["out"]               # [2, 32*256]
        # pair layout: col = 512*pr + 256*ii + j, local row = 16*ii + pr
        logits[IBLK * core:IBLK * core + IBLK] = \
            o.reshape(2, 16, 2, B).transpose(2, 1, 3, 0).reshape(IBLK, B, 2)
    logits += inputs["b2"].astype(np.float32)[None, None, :]
    pos = logits[np.arange(B), np.arange(B)]
    qi, ri = np.nonzero(~np.eye(B, dtype=bool))
    neg = logits[qi, ri]
    return np.concatenate([pos, neg], axis=0).astype(np.float32)


if __name__ == "__main__":
    _build()
    print("build OK")


# revision 29
# speedup vs baseline: 1.2339x; 1.0077x over previous
"""Trainium2 Bass kernel for nn_BestModel5 (dual-GRU encoder + BxB pair classifier).

Sharding (8 cores): cores 0-3 query-GRU batch shards of 64; cores 4-7 reply-GRU.
Classifier sharded 8-way over the 256 query rows (32 i-rows/core).
Embedding gather + layout prep on host; all matmuls bf16 on PE, f32 PSUM.

GRU step pipeline: x-projections (incl. length-mask row and a ones-row that
carries the gate biases, both zero-padded to K=128 - a short stationary
operand pins LDWEIGHTS to row-group q0 and serializes the PE) are
matmul-accumulated straight into PSUM two steps per bank, so the recurrent
chain is just
  h-MMs -> sigmoid(PSUM) -> r*h -> cand-MMs -> tanh(PSUM) -> zbar*c -> + z*h
with all elementwise ops in bf16 (2x DVE mode) and z*h = h - zbar*h built
off-chain from sigmoid(-zpre).
"""

import numpy as np
import ml_dtypes

BF16 = ml_dtypes.bfloat16

V, E, H, B, T = 100000, 256, 256, 256, 40
D_HID, D_OUT = 256, 2
NCORES = 8
BSH = 64            # batch rows per GRU shard
NSH = 4             # GRU batch shards per encoder
BT = BSH * T        # 2560 columns of xembT per core
IBLK = B // NCORES  # 32 classifier i-rows per core
NPAIR = T // 2      # PSUM step-pairs

_cache = {}


def _build(sim_gelu=False):
    """Build + compile the SPMD Bass program once."""
    import concourse.bacc as bacc
    import concourse.bass as bass
    import concourse.tile as tile
    import concourse.mybir as mybir

    f32 = mybir.dt.float32
    bf16 = mybir.dt.bfloat16
    AF = mybir.ActivationFunctionType

    nc = bacc.Bacc("TRN2", target_bir_lowering=False, debug=False, num_devices=NCORES)

    def din(name, shape, dt):
        return nc.dram_tensor(name, shape, dt, kind="ExternalInput").ap()

    # per-core inputs (content differs per core; shapes identical)
    xembT = din("xembT", [E + 2, BT], bf16)      # rows 0-255 emb, 256 mask, 257 ones
    whg = din("whg", [H, 2 * H], bf16)           # Wg[E:E+H, :]
    wxg = din("wxg", [E, 2 * H], bf16)           # Wg[:E, :]
    wxgm = din("wxgm", [2, 2 * H], bf16)         # row0 = [0|30] mask row, row1 = bg
    wch = din("wch", [H, H], bf16)               # Wc[E:E+H, :]
    wxc = din("wxc", [E, H], bf16)               # Wc[:E, :]
    wxcm = din("wxcm", [2, H], bf16)             # row0 = 0, row1 = bc
    w1q = din("w1q", [H, D_HID], bf16)           # W1[:256]
    w1r = din("w1r", [H, D_HID], bf16)           # W1[257:513]
    wdt = din("wdt", [1, IBLK // 2 * D_HID], bf16)  # W1[256] tiled 16x
    rhsb = din("rhsb", [4, IBLK * B], bf16)      # [0;ones|0;0|0;0;ones] pattern
    b1 = din("b1", [D_HID], f32)
    w2 = din("w2", [D_HID, D_OUT], bf16)

    out = nc.dram_tensor("out", [D_OUT, IBLK * B], f32, kind="ExternalOutput").ap()

    with tile.TileContext(nc) as tc:
        with (
            tc.tile_pool(name="persist", bufs=1) as pp,
            tc.tile_pool(name="dram", bufs=1, space="DRAM") as dramp,
        ):
            # ---- loads: chunk 0 + x-weights first so the recurrence can
            # start immediately; h-weights next; classifier weights deferred ----
            xT = [pp.tile([128, BT], bf16, tag=f"xT{k}", name=f"xT{k}") for k in range(2)]
            xTm = pp.tile([128, BT], bf16, tag="xTm", name="xTm")
            nc.vector.memset(xTm[:, 0:256], 0.0)

            cs0 = slice(0, 256)
            nc.sync.dma_start(xT[0][:, cs0], xembT[0:128, cs0])
            nc.gpsimd.dma_start(xT[1][:, cs0], xembT[128:256, cs0])
            nc.scalar.dma_start(xTm[0:2, cs0], xembT[256:258, cs0])

            wxgm_s = pp.tile([128, 2 * H], bf16, tag="wxgm", name="wxgm")
            nc.vector.memset(wxgm_s[:], 0.0)
            wxcm_s = pp.tile([128, H], bf16, tag="wxcm", name="wxcm")
            nc.vector.memset(wxcm_s[:], 0.0)
            nc.vector.memset(xTm[:, 256:BT], 0.0)

            wxg_s = [pp.tile([128, 2 * H], bf16, tag=f"wxg{k}", name=f"wxg{k}") for k in range(2)]
            nc.sync.dma_start(wxg_s[0][:], wxg[0:128, :])
            nc.gpsimd.dma_start(wxg_s[1][:], wxg[128:256, :])
            nc.scalar.dma_start(wxgm_s[0:2, :], wxgm[:])
            wxc_s = [pp.tile([128, H], bf16, tag=f"wxc{k}", name=f"wxc{k}") for k in range(2)]
            nc.sync.dma_start(wxc_s[0][:], wxc[0:128, :])
            nc.gpsimd.dma_start(wxc_s[1][:], wxc[128:256, :])
            nc.scalar.dma_start(wxcm_s[0:2, :], wxcm[:])

            whg_s = [pp.tile([128, 2 * H], bf16, tag=f"whg{k}", name=f"whg{k}") for k in range(2)]
            nc.sync.dma_start(whg_s[0][:], whg[0:128, :])
            nc.gpsimd.dma_start(whg_s[1][:], whg[128:256, :])
            wch_s = [pp.tile([128, H], bf16, tag=f"wch{k}", name=f"wch{k}") for k in range(2)]
            nc.sync.dma_start(wch_s[0][:], wch[0:128, :])
            nc.gpsimd.dma_start(wch_s[1][:], wch[128:256, :])

            # preload sigmoid/tanh ACT table during DMA wait
            warm_in = pp.tile([1, 2], f32, tag="warmi", name="warmi")
            nc.vector.memset(warm_in[:], 0.0)
            warm_out = pp.tile([1, 2], f32, tag="warmo", name="warmo")
            nc.scalar.activation(warm_out[:], warm_in[:], AF.Sigmoid)

            # remaining xembT in 256-col (2-pair) chunks, in step order
            dq = [nc.sync, nc.gpsimd, nc.scalar]
            for J in range(1, NPAIR // 2):
                cs = slice(256 * J, 256 * J + 256)
                dq[J % 3].dma_start(xT[0][:, cs], xembT[0:128, cs])
                dq[(J + 1) % 3].dma_start(xT[1][:, cs], xembT[128:256, cs])
                dq[(J + 2) % 3].dma_start(xTm[0:2, cs], xembT[256:258, cs])

            # classifier static operands (pad rows 4-127 with zeros so the
            # stationary operand spans all PE row groups)
            lhs4 = pp.tile([128, IBLK // 2 * D_HID], bf16, tag="lhs4", name="lhs4")
            nc.vector.memset(lhs4[:], 0.0)
            rhs4 = pp.tile([128, IBLK * B], bf16, tag="rhs4", name="rhs4")
            nc.vector.memset(rhs4[:], 0.0)

            # ---- GRU recurrence ----
            with (
                tc.tile_pool(name="grpsum", bufs=2, space="PSUM") as grp,
                tc.tile_pool(name="gzpsum", bufs=2, space="PSUM") as gzp,
                tc.tile_pool(name="cpsum", bufs=3, space="PSUM") as cp,
                tc.tile_pool(name="step", bufs=6) as sp,
            ):
                def emit_gx(j, pool, mbase, tag):
                    """Gates x-part (2 m-blocks) for pair j; cols 128*m+64*tau+b."""
                    g = pool.tile([128, 512], f32, tag=tag, name=f"{tag}{j}")
                    cs = slice(128 * j, 128 * j + 128)
                    for mi in range(2):
                        m = mbase + mi
                        ms = slice(128 * m, 128 * m + 128)
                        os = slice(128 * mi, 128 * mi + 128)
                        nc.tensor.matmul(g[:, os], wxg_s[0][:, ms], xT[0][:, cs],
                                         start=(mi == 0), stop=False)
                        nc.tensor.matmul(g[:, os], wxg_s[1][:, ms], xT[1][:, cs],
                                         start=False, stop=False)
                        nc.tensor.matmul(g[:, os], wxgm_s[:, ms], xTm[:, cs],
                                         start=False, stop=False)
                    return g

                def emit_cx(j, mi):
                    """Cand x-part m-block mi for pair j (new tile when mi=0)."""
                    if mi == 0:
                        c = cp.tile([128, 512], f32, tag="c", name=f"c{j}")
                        emit_cx.cur = c
                    c = emit_cx.cur
                    cs = slice(128 * j, 128 * j + 128)
                    ms = slice(128 * mi, 128 * mi + 128)
                    nc.tensor.matmul(c[:, ms], wxc_s[0][:, ms], xT[0][:, cs],
                                     start=(mi == 0), stop=False)
                    nc.tensor.matmul(c[:, ms], wxc_s[1][:, ms], xT[1][:, cs],
                                     start=False, stop=False)
                    nc.tensor.matmul(c[:, ms], wxcm_s[:, ms], xTm[:, cs],
                                     start=False, stop=False)
                    return c

                h_bf = pp.tile([128, 128], bf16, tag="hbf", name="hbf", bufs=4)
                nc.vector.memset(h_bf[:], 0.0)

                gr_cur = emit_gx(0, grp, 0, "gr")
                gz_cur = emit_gx(0, gzp, 2, "gz")
                emit_cx(0, 0)
                c_cur = emit_cx(0, 1)
                gr_nxt = gz_nxt = c_nxt = None

                for t in range(T):
                    j, tau = t // 2, t % 2
                    off = 64 * tau

                    # gates h-part: r then z
                    for mi in range(2):
                        for k in range(2):
                            last = (tau == 1 and mi == 1 and k == 1)
                            nc.tensor.matmul(
                                gr_cur[:, 128 * mi + off:128 * mi + off + 64],
                                whg_s[k][:, 128 * mi:128 * mi + 128],
                                h_bf[:, 64 * k:64 * k + 64],
                                start=False, stop=last)
                    for mi in range(2):
                        for k in range(2):
                            last = (tau == 1 and mi == 1 and k == 1)
                            nc.tensor.matmul(
                                gz_cur[:, 128 * mi + off:128 * mi + off + 64],
                                whg_s[k][:, 128 * (mi + 2):128 * (mi + 2) + 128],
                                h_bf[:, 64 * k:64 * k + 64],
                                start=False, stop=last)

                    # x-batch A fills the PE while sigmoid+mul run
                    if j + 1 < NPAIR:
                        if tau == 0:
                            gr_nxt = emit_gx(j + 1, grp, 0, "gr")
                        else:
                            emit_cx(j + 1, 0)

                    gr_v = gr_cur[:, 0:256].rearrange("p (m x) -> p m x", m=2, x=128)
                    gz_v = gz_cur[:, 0:256].rearrange("p (m x) -> p m x", m=2, x=128)
                    r_bf = sp.tile([128, 128], bf16, tag="r", name="r")
                    nc.scalar.activation(
                        r_bf[:].rearrange("p (m b) -> p m b", m=2, b=64),
                        gr_v[:, :, off:off + 64], AF.Sigmoid)
                    zb_bf = sp.tile([128, 128], bf16, tag="zb", name="zb")
                    nc.scalar.activation(
                        zb_bf[:].rearrange("p (m b) -> p m b", m=2, b=64),
                        gz_v[:, :, off:off + 64], AF.Sigmoid, scale=-1.0)

                    rh = sp.tile([128, 128], bf16, tag="rh", name="rh")
                    nc.vector.tensor_mul(rh[:], r_bf[:], h_bf[:])

                    # cand h-part
                    for mi in range(2):
                        for k in range(2):
                            last = (tau == 1 and mi == 1 and k == 1)
                            nc.tensor.matmul(
                                c_cur[:, 128 * mi + off:128 * mi + off + 64],
                                wch_s[k][:, 128 * mi:128 * mi + 128],
                                rh[:, 64 * k:64 * k + 64],
                                start=False, stop=last)

                    # x-batch B in the shadow of tanh + tail
                    if j + 1 < NPAIR:
                        if tau == 0:
                            gz_nxt = emit_gx(j + 1, gzp, 2, "gz")
                        else:
                            c_nxt = emit_cx(j + 1, 1)

                    # z*h = h - zbar*h, off the critical chain
                    s1 = sp.tile([128, 128], bf16, tag="s1", name="s1")
                    nc.vector.tensor_mul(s1[:], zb_bf[:], h_bf[:])
                    hd = sp.tile([128, 128], bf16, tag="hd", name="hd")
                    nc.vector.tensor_sub(hd[:], h_bf[:], s1[:])

                    c_v = c_cur[:, 0:256].rearrange("p (m x) -> p m x", m=2, x=128)
                    c_bf = sp.tile([128, 128], bf16, tag="ct", name="ct")
                    nc.scalar.activation(
                        c_bf[:].rearrange("p (m b) -> p m b", m=2, b=64),
                        c_v[:, :, off:off + 64], AF.Tanh)

                    zbc = sp.tile([128, 128], bf16, tag="zbc", name="zbc")
                    nc.vector.tensor_mul(zbc[:], zb_bf[:], c_bf[:])
                    h_new = pp.tile([128, 128], bf16, tag="hbf", name="hbf", bufs=4)
                    nc.vector.tensor_add(h_new[:], zbc[:], hd[:])
                    h_bf = h_new

                    if tau == 1 and j + 1 < NPAIR:
                        gr_cur, gz_cur, c_cur = gr_nxt, gz_nxt, c_nxt

            # ---- exchange encodings ----
            ag_in = dramp.tile([128, 128], bf16, tag="agin", name="agin")
            ag_out = dramp.tile([NCORES, 128, 128], bf16, tag="agout", name="agout")

            nc.sync.dma_start(ag_in[:], h_bf[:])
            nc.gpsimd.collective_compute(
                "AllGather", mybir.AluOpType.bypass,
                replica_groups=[list(range(NCORES))],
                ins=[ag_in.opt()], outs=[ag_out.opt()])

            # classifier weights + gelu ACT table load overlap the collective
            w1q_s = [pp.tile([128, D_HID], bf16, tag=f"w1q{k}", name=f"w1q{k}") for k in range(2)]
            nc.sync.dma_start(w1q_s[0][:], w1q[0:128, :])
            nc.sync.dma_start(w1q_s[1][:], w1q[128:256, :])
            w1r_s = [pp.tile([128, D_HID], bf16, tag=f"w1r{k}", name=f"w1r{k}") for k in range(2)]
            nc.gpsimd.dma_start(w1r_s[0][:], w1r[0:128, :])
            nc.gpsimd.dma_start(w1r_s[1][:], w1r[128:256, :])
            b1_s = pp.tile([128, 2], f32, tag="b1", name="b1")
            nc.scalar.dma_start(b1_s[:], b1.rearrange("(m p) -> p m", p=128))
            w2_s = [pp.tile([128, D_OUT], bf16, tag=f"w2{k}", name=f"w2{k}") for k in range(2)]
            nc.scalar.dma_start(w2_s[0][:], w2[0:128, :])
            nc.scalar.dma_start(w2_s[1][:], w2[128:256, :])
            nc.sync.dma_start(lhs4[0:1, :], wdt[:])
            nc.sync.dma_start(lhs4[2:3, :], wdt[:])
            nc.gpsimd.dma_start(rhs4[0:4, :], rhsb[:])

            gelu_af = mybir.ActivationFunctionType.Tanh if sim_gelu \
                else mybir.ActivationFunctionType.Gelu_apprx_tanh
            warm_out2 = pp.tile([1, 2], f32, tag="warmo2", name="warmo2")
            nc.scalar.activation(warm_out2[:], warm_in[:], gelu_af)

            # per-core q slice: rows [32*core, 32*core+32) live on gathered
            # block core//2, batch-half core%2 -> one register-offset DMA,
            # issued first since the q1->lhs4 staging chain hangs off it
            qloc = pp.tile([128, 2 * IBLK], bf16, tag="qloc", name="qloc")  # [p, c*32+i]
            pid = nc.sync.partition_id()
            agv = ag_out.rearrange("n p (c h b) -> n p c h b", c=2, h=2, b=32)
            nc.sync.dma_start(
                qloc[:].rearrange("p (c b) -> p c b", c=2, b=32),
                agv[bass.ds(pid >> 1, 1), :, :, bass.ds(pid & 1, 1), :])

            # rT2: whole gathered r blocks; col = 128*kshard + 64*chalf + b
            rT2 = pp.tile([128, 512], bf16, tag="rT2", name="rT2")
            qeng = [nc.gpsimd, nc.scalar, nc.sync, nc.gpsimd]
            for k in range(NSH):
                qeng[k].dma_start(rT2[:, 128 * k:128 * k + 128], ag_out[4 + k])
            # per-hidden-half view with j = 64*kshard + b ordering
            rT_v = rT2[:].rearrange("p (k c b) -> p c k b", k=4, c=2, b=64)

            # ---- classifier ----
            with (
                tc.tile_pool(name="spsum", bufs=1, space="PSUM") as sps,
                tc.tile_pool(name="hpsum", bufs=2, space="PSUM") as hps,
                tc.tile_pool(name="lpsum", bufs=1, space="PSUM") as lps,
                tc.tile_pool(name="cls", bufs=3) as cpool,
            ):
                # PE warm-up: the HAM clock-gate leaves the array at 1.2 GHz
                # after the long collective idle; junk matmuls spanning the
                # preamble restore 2.4 GHz before the main classifier stream
                def warm(n):
                    for _ in range(n):
                        jnk = hps.tile([128, 4 * B], f32, tag="hps", name="hps")
                        nc.tensor.matmul(jnk[:, 0:512], whg_s[0][:, 0:128],
                                         rT2[:], start=True, stop=True)
                    return jnk

                # R1T + b1 first: only needs rT2, overlaps the qloc chain
                r1tb = pp.tile([128, 2 * B], f32, tag="r1tb", name="r1tb")
                for m in range(2):
                    ps3 = sps.tile([128, B], f32, tag="sps", name="sps")
                    for k in range(2):
                        nc.tensor.matmul(ps3[:],
                                         w1r_s[k][:, 128 * m:128 * m + 128],
                                         rT_v[:, k],
                                         start=(k == 0), stop=(k == 1))
                    nc.scalar.activation(r1tb[:, 256 * m:256 * m + 256], ps3[:],
                                         AF.Identity, bias=b1_s[:, m:m + 1])

                # Q1 rows for my i's: [32, 256] bf16
                ps = sps.tile([IBLK, D_HID], f32, tag="sps", name="sps")
                for c in range(2):
                    nc.tensor.matmul(ps[:], qloc[:, 32 * c:32 * c + 32],
                                     w1q_s[c][:], start=(c == 0), stop=(c == 1))
                q1 = pp.tile([IBLK, D_HID], bf16, tag="q1", name="q1")
                nc.scalar.activation(q1[:], ps[:], AF.Copy, bias=0.0)
                nc.sync.dma_start(lhs4[1:2, :], q1[0:16, :])
                nc.gpsimd.dma_start(lhs4[3:4, :], q1[16:32, :])

                # dist rows for my i's: [32, 256] bf16
                ps2 = sps.tile([IBLK, B], f32, tag="sps", name="sps")
                for c in range(2):
                    nc.tensor.matmul(ps2[:], qloc[:, 32 * c:32 * c + 32],
                                     rT_v[:, c], start=(c == 0), stop=(c == 1))
                dist = pp.tile([IBLK, B], bf16, tag="dist", name="dist")
                nc.scalar.activation(dist[:], ps2[:], AF.Copy, bias=0.0)
                nc.gpsimd.dma_start(
                    rhs4[0:1, :].rearrange("o (p ii j) -> o p ii j",
                                           p=IBLK // 2, ii=2, j=B)[:, :, 0, :],
                    dist[0:16, :])
                nc.sync.dma_start(
                    rhs4[2:3, :].rearrange("o (p ii j) -> o p ii j",
                                           p=IBLK // 2, ii=2, j=B)[:, :, 1, :],
                    dist[16:32, :])

                r1tb2 = pp.tile([128, 4 * B], f32, tag="r1tb2", name="r1tb2")
                r2v = r1tb2[:].rearrange("p (m ii j) -> p m ii j", m=2, ii=2,
                                         j=B)
                for ii in range(2):
                    nc.vector.tensor_copy(
                        r2v[:, :, ii, :],
                        r1tb[:].rearrange("p (m j) -> p m j", m=2, j=B))

                jnk = warm(12)
                warm_sb = pp.tile([128, 4], f32, tag="warm3", name="warm3")
                nc.vector.tensor_copy(warm_sb[:], jnk[:, 0:4])
                warm_dram = dramp.tile([128, 4], f32, tag="warmd", name="warmd")
                nc.scalar.dma_start(warm_dram[:], warm_sb[:])

                out_sb = pp.tile([D_OUT, IBLK * B], f32, tag="outsb", name="outsb")
                # pair-rows processed two at a time so gelu and the output
                # copy amortize their fixed cost over 1024 columns; the output
                # copy alternates Scalar/Vector to balance engine load (b2 is
                # added on the host)
                # r1tb4: r1 operand duplicated over (m, sub) for 1024-wide adds
                r1tb4 = pp.tile([128, 8 * B], f32, tag="r1tb4", name="r1tb4")
                for m in range(2):
                    for sub in range(2):
                        nc.vector.tensor_copy(
                            r1tb4[:, 1024 * m + 512 * sub:
                                  1024 * m + 512 * sub + 512],
                            r1tb2[:, 512 * m:512 * m + 512])

                def emit_w2(prp, h1):
                    """Second layer + output copy for pair-row-pair prp."""
                    l_ps = lps.tile([D_OUT, 4 * B], f32, tag="lps", name="lps")
                    for sub in range(2):
                        for k in range(2):
                            nc.tensor.matmul(
                                l_ps[:, 512 * sub:512 * sub + 512], w2_s[k][:],
                                h1[:, 1024 * k + 512 * sub:
                                   1024 * k + 512 * sub + 512],
                                start=(k == 0), stop=(k == 1))
                    oseg = out_sb[:, 1024 * prp:1024 * prp + 1024]
                    if prp % 2 == 0:
                        nc.scalar.activation(oseg, l_ps[:], AF.Copy, bias=0.0)
                    else:
                        nc.vector.tensor_copy(oseg, l_ps[:])
                    [nc.sync, nc.gpsimd][prp % 2].dma_start(
                        out[:, 1024 * prp:1024 * prp + 1024], oseg)

                # software-pipelined: W2(prp-1) is emitted after the h1 MMs of
                # prp so the in-order PE never stalls on gelu(prp)
                prev = None
                for prp in range(IBLK // 4):
                    pr0 = 2 * prp
                    # h1 block layout: col = 1024*m + 512*sub + 256*ii + j
                    h1p4 = cpool.tile([128, 8 * B], bf16, tag="h1p", name="h1p")
                    for m in range(2):
                        h_ps = hps.tile([128, 4 * B], f32, tag="hps",
                                        name="hps")
                        for sub in range(2):
                            pr = pr0 + sub
                            nc.tensor.matmul(
                                h_ps[:, 512 * sub:512 * sub + 512],
                                lhs4[0:128, D_HID * pr + 128 * m:
                                     D_HID * pr + 128 * m + 128],
                                rhs4[0:128, 2 * B * pr:2 * B * pr + 2 * B],
                                start=True, stop=True)
                        nc.vector.tensor_add(
                            h1p4[:, 1024 * m:1024 * m + 1024], h_ps[:],
                            r1tb4[:, 1024 * m:1024 * m + 1024])
                    h1 = cpool.tile([128, 8 * B], bf16, tag="h1", name="h1")
                    nc.scalar.activation(h1[:], h1p4[:], gelu_af)
                    if prev is not None:
                        emit_w2(prev[0], prev[1])
                    prev = (prp, h1)
                emit_w2(prev[0], prev[1])

    nc.compile()
    return nc


def _rhs_base():
    """[4, IBLK*B] pattern: per 512-col pair-block rows are
    [0,0],[ones,0],[0,0],[0,ones] - dist blocks get DMA'd in on device."""
    r = np.zeros((4, IBLK * B), dtype=BF16)
    v = r.reshape(4, IBLK // 2, 2, B)
    v[1, :, 0, :] = 1.0
    v[3, :, 1, :] = 1.0
    return r


def _prep_inputs(inputs):
    """Host-side prep: embed+transpose sequences, split weights, per-core maps."""
    emb = inputs["embeddings"]
    in_maps = []
    f32 = np.float32

    # classifier tensors (identical on all cores)
    W1, b1, W2, b2 = (inputs["W1"], inputs["b1"], inputs["W2"], inputs["b2"])
    common = {
        "w1q": np.ascontiguousarray(W1[:H]).astype(BF16),
        "w1r": np.ascontiguousarray(W1[H + 1:]).astype(BF16),
        "wdt": np.tile(np.ascontiguousarray(W1[H:H + 1]).astype(BF16),
                       (1, IBLK // 2)),
        "rhsb": _rhs_base(),
        "b1": b1.astype(f32),
        "w2": W2.astype(BF16),
    }

    ones_row = np.ones((1, BT), f32)
    for core in range(NCORES):
        enc = core // NSH
        s = core % NSH
        if enc == 0:
            seqs, lens = inputs["input_queries"], inputs["query_lengths"]
            Wg, bgv, Wc, bcv = (inputs["Wg_q"], inputs["bg_q"],
                                inputs["Wc_q"], inputs["bc_q"])
        else:
            seqs, lens = inputs["input_replies"], inputs["reply_lengths"]
            Wg, bgv, Wc, bcv = (inputs["Wg_r"], inputs["bg_r"],
                                inputs["Wc_r"], inputs["bc_r"])
        rows = slice(BSH * s, BSH * s + BSH)
        xe = emb[seqs[rows]]                       # [64, 40, 256]
        xT = np.transpose(xe, (2, 1, 0)).reshape(E, BT)  # col = t*64+b
        lmask = (np.arange(T)[:, None] >= lens[rows][None, :]) \
            .astype(f32).reshape(1, BT)
        xembT = np.concatenate([xT, lmask, ones_row], axis=0).astype(BF16)

        mask_row = np.concatenate([np.zeros(H, f32), np.full(H, 30.0, f32)])
        wxgm = np.stack([mask_row, bgv.astype(f32)]).astype(BF16)
        wxcm = np.stack([np.zeros(H, f32), bcv.astype(f32)]).astype(BF16)

        m = {
            "xembT": xembT,
            "whg": np.ascontiguousarray(Wg[E:]).astype(BF16),
            "wxg": np.ascontiguousarray(Wg[:E]).astype(BF16),
            "wxgm": wxgm,
            "wch": np.ascontiguousarray(Wc[E:]).astype(BF16),
            "wxc": np.ascontiguousarray(Wc[:E]).astype(BF16),
            "wxcm": wxcm,
        }
        m.update(common)
        in_maps.append(m)
    return in_maps


def run_cores(in_maps, trace=False):
    from concourse.bass_utils import run_bass_kernel_spmd
    from concourse.bass_interp import get_hw_module

    if "nc" not in _cache:
        _cache["nc"] = _build()
    nc = _cache["nc"]
    old = nc.m
    nc.m = _cache.setdefault("hwm", get_hw_module(nc.m))
    try:
        res = run_bass_kernel_spmd(nc, in_maps, core_ids=list(range(NCORES)),
                                   trace=trace)
    finally:
        nc.m = old
    return res


def kernel(**inputs):
    in_maps = _prep_inputs(inputs)
    res = run_cores(in_maps)
    logits = np.zeros((B, B, 2), np.float32)
    for core in range(NCORES):
        o = res.results[core]["out"]               # [2, 32*256]
        # pair layout: col = 512*pr + 256*ii + j, local row = 16*ii + pr
        logits[IBLK * core:IBLK * core + IBLK] = \
            o.reshape(2, 16, 2, B).transpose(2, 1, 3, 0).reshape(IBLK, B, 2)
    logits += inputs["b2"].astype(np.float32)[None, None, :]
    pos = logits[np.arange(B), np.arange(B)]
    qi, ri = np.nonzero(~np.eye(B, dtype=bool))
    neg = logits[qi, ri]
    return np.concatenate([pos, neg], axis=0).astype(np.float32)


if __name__ == "__main__":
    _build()
    print("build OK")


# revision 30
# speedup vs baseline: 1.2369x; 1.0025x over previous
"""Trainium2 Bass kernel for nn_BestModel5 (dual-GRU encoder + BxB pair classifier).

Sharding (8 cores): cores 0-3 query-GRU batch shards of 64; cores 4-7 reply-GRU.
Classifier sharded 8-way over the 256 query rows (32 i-rows/core).
Embedding gather + layout prep on host; all matmuls bf16 on PE, f32 PSUM.

GRU step pipeline: x-projections (incl. length-mask row and a ones-row that
carries the gate biases, both zero-padded to K=128 - a short stationary
operand pins LDWEIGHTS to row-group q0 and serializes the PE) are
matmul-accumulated straight into PSUM two steps per bank, so the recurrent
chain is just
  h-MMs -> sigmoid(PSUM) -> r*h -> cand-MMs -> tanh(PSUM) -> zbar*c -> + z*h
with all elementwise ops in bf16 (2x DVE mode) and z*h = h - zbar*h built
off-chain from sigmoid(-zpre).
"""

import numpy as np
import ml_dtypes

BF16 = ml_dtypes.bfloat16

V, E, H, B, T = 100000, 256, 256, 256, 40
D_HID, D_OUT = 256, 2
NCORES = 8
BSH = 64            # batch rows per GRU shard
NSH = 4             # GRU batch shards per encoder
BT = BSH * T        # 2560 columns of xembT per core
IBLK = B // NCORES  # 32 classifier i-rows per core
NPAIR = T // 2      # PSUM step-pairs

_cache = {}


def _build(sim_gelu=False):
    """Build + compile the SPMD Bass program once."""
    import concourse.bacc as bacc
    import concourse.bass as bass
    import concourse.tile as tile
    import concourse.mybir as mybir

    f32 = mybir.dt.float32
    bf16 = mybir.dt.bfloat16
    AF = mybir.ActivationFunctionType

    nc = bacc.Bacc("TRN2", target_bir_lowering=False, debug=False, num_devices=NCORES)

    def din(name, shape, dt):
        return nc.dram_tensor(name, shape, dt, kind="ExternalInput").ap()

    # per-core inputs (content differs per core; shapes identical)
    xembT = din("xembT", [E + 2, BT], bf16)      # rows 0-255 emb, 256 mask, 257 ones
    whg = din("whg", [H, 2 * H], bf16)           # Wg[E:E+H, :]
    wxg = din("wxg", [E, 2 * H], bf16)           # Wg[:E, :]
    wxgm = din("wxgm", [2, 2 * H], bf16)         # row0 = [0|30] mask row, row1 = bg
    wch = din("wch", [H, H], bf16)               # Wc[E:E+H, :]
    wxc = din("wxc", [E, H], bf16)               # Wc[:E, :]
    wxcm = din("wxcm", [2, H], bf16)             # row0 = 0, row1 = bc
    w1q = din("w1q", [H, D_HID], bf16)           # W1[:256]
    w1r = din("w1r", [H, D_HID], bf16)           # W1[257:513]
    wdt = din("wdt", [1, IBLK // 2 * D_HID], bf16)  # W1[256] tiled 16x
    rhsb = din("rhsb", [4, IBLK * B], bf16)      # [0;ones|0;0|0;0;ones] pattern
    b1 = din("b1", [D_HID], f32)
    w2 = din("w2", [D_HID, D_OUT], bf16)

    out = nc.dram_tensor("out", [D_OUT, IBLK * B], f32, kind="ExternalOutput").ap()

    with tile.TileContext(nc) as tc:
        with (
            tc.tile_pool(name="persist", bufs=1) as pp,
            tc.tile_pool(name="dram", bufs=1, space="DRAM") as dramp,
        ):
            # ---- loads: chunk 0 + x-weights first so the recurrence can
            # start immediately; h-weights next; classifier weights deferred ----
            xT = [pp.tile([128, BT], bf16, tag=f"xT{k}", name=f"xT{k}") for k in range(2)]
            xTm = pp.tile([128, BT], bf16, tag="xTm", name="xTm")
            nc.vector.memset(xTm[:, 0:256], 0.0)

            cs0 = slice(0, 256)
            nc.sync.dma_start(xT[0][:, cs0], xembT[0:128, cs0])
            nc.gpsimd.dma_start(xT[1][:, cs0], xembT[128:256, cs0])
            nc.scalar.dma_start(xTm[0:2, cs0], xembT[256:258, cs0])

            wxgm_s = pp.tile([128, 2 * H], bf16, tag="wxgm", name="wxgm")
            nc.vector.memset(wxgm_s[:], 0.0)
            wxcm_s = pp.tile([128, H], bf16, tag="wxcm", name="wxcm")
            nc.vector.memset(wxcm_s[:], 0.0)
            nc.vector.memset(xTm[:, 256:BT], 0.0)

            wxg_s = [pp.tile([128, 2 * H], bf16, tag=f"wxg{k}", name=f"wxg{k}") for k in range(2)]
            nc.sync.dma_start(wxg_s[0][:], wxg[0:128, :])
            nc.gpsimd.dma_start(wxg_s[1][:], wxg[128:256, :])
            nc.scalar.dma_start(wxgm_s[0:2, :], wxgm[:])
            wxc_s = [pp.tile([128, H], bf16, tag=f"wxc{k}", name=f"wxc{k}") for k in range(2)]
            nc.sync.dma_start(wxc_s[0][:], wxc[0:128, :])
            nc.gpsimd.dma_start(wxc_s[1][:], wxc[128:256, :])
            nc.scalar.dma_start(wxcm_s[0:2, :], wxcm[:])

            whg_s = [pp.tile([128, 2 * H], bf16, tag=f"whg{k}", name=f"whg{k}") for k in range(2)]
            nc.sync.dma_start(whg_s[0][:], whg[0:128, :])
            nc.gpsimd.dma_start(whg_s[1][:], whg[128:256, :])
            wch_s = [pp.tile([128, H], bf16, tag=f"wch{k}", name=f"wch{k}") for k in range(2)]
            nc.sync.dma_start(wch_s[0][:], wch[0:128, :])
            nc.gpsimd.dma_start(wch_s[1][:], wch[128:256, :])

            # preload sigmoid/tanh ACT table during DMA wait
            warm_in = pp.tile([1, 2], f32, tag="warmi", name="warmi")
            nc.vector.memset(warm_in[:], 0.0)
            warm_out = pp.tile([1, 2], f32, tag="warmo", name="warmo")
            nc.scalar.activation(warm_out[:], warm_in[:], AF.Sigmoid)

            # remaining xembT in 256-col (2-pair) chunks, in step order
            dq = [nc.sync, nc.gpsimd, nc.scalar]
            for J in range(1, NPAIR // 2):
                cs = slice(256 * J, 256 * J + 256)
                dq[J % 3].dma_start(xT[0][:, cs], xembT[0:128, cs])
                dq[(J + 1) % 3].dma_start(xT[1][:, cs], xembT[128:256, cs])
                dq[(J + 2) % 3].dma_start(xTm[0:2, cs], xembT[256:258, cs])

            # classifier static operands (pad rows 4-127 with zeros so the
            # stationary operand spans all PE row groups)
            lhs4 = pp.tile([128, IBLK // 2 * D_HID], bf16, tag="lhs4", name="lhs4")
            nc.vector.memset(lhs4[:], 0.0)
            rhs4 = pp.tile([128, IBLK * B], bf16, tag="rhs4", name="rhs4")
            nc.vector.memset(rhs4[:], 0.0)

            # ---- GRU recurrence ----
            with (
                tc.tile_pool(name="grpsum", bufs=2, space="PSUM") as grp,
                tc.tile_pool(name="gzpsum", bufs=2, space="PSUM") as gzp,
                tc.tile_pool(name="cpsum", bufs=3, space="PSUM") as cp,
                tc.tile_pool(name="step", bufs=6) as sp,
            ):
                def emit_gx(j, pool, mbase, tag):
                    """Gates x-part (2 m-blocks) for pair j; cols 128*m+64*tau+b."""
                    g = pool.tile([128, 512], f32, tag=tag, name=f"{tag}{j}")
                    cs = slice(128 * j, 128 * j + 128)
                    for mi in range(2):
                        m = mbase + mi
                        ms = slice(128 * m, 128 * m + 128)
                        os = slice(128 * mi, 128 * mi + 128)
                        nc.tensor.matmul(g[:, os], wxg_s[0][:, ms], xT[0][:, cs],
                                         start=(mi == 0), stop=False)
                        nc.tensor.matmul(g[:, os], wxg_s[1][:, ms], xT[1][:, cs],
                                         start=False, stop=False)
                        nc.tensor.matmul(g[:, os], wxgm_s[:, ms], xTm[:, cs],
                                         start=False, stop=False)
                    return g

                def emit_cx(j, mi):
                    """Cand x-part m-block mi for pair j (new tile when mi=0)."""
                    if mi == 0:
                        c = cp.tile([128, 512], f32, tag="c", name=f"c{j}")
                        emit_cx.cur = c
                    c = emit_cx.cur
                    cs = slice(128 * j, 128 * j + 128)
                    ms = slice(128 * mi, 128 * mi + 128)
                    nc.tensor.matmul(c[:, ms], wxc_s[0][:, ms], xT[0][:, cs],
                                     start=(mi == 0), stop=False)
                    nc.tensor.matmul(c[:, ms], wxc_s[1][:, ms], xT[1][:, cs],
                                     start=False, stop=False)
                    nc.tensor.matmul(c[:, ms], wxcm_s[:, ms], xTm[:, cs],
                                     start=False, stop=False)
                    return c

                h_bf = pp.tile([128, 128], bf16, tag="hbf", name="hbf", bufs=4)
                nc.vector.memset(h_bf[:], 0.0)

                gr_cur = emit_gx(0, grp, 0, "gr")
                gz_cur = emit_gx(0, gzp, 2, "gz")
                emit_cx(0, 0)
                c_cur = emit_cx(0, 1)
                gr_nxt = gz_nxt = c_nxt = None

                for t in range(T):
                    j, tau = t // 2, t % 2
                    off = 64 * tau

                    # gates h-part: r then z
                    for mi in range(2):
                        for k in range(2):
                            last = (tau == 1 and mi == 1 and k == 1)
                            nc.tensor.matmul(
                                gr_cur[:, 128 * mi + off:128 * mi + off + 64],
                                whg_s[k][:, 128 * mi:128 * mi + 128],
                                h_bf[:, 64 * k:64 * k + 64],
                                start=False, stop=last)
                    for mi in range(2):
                        for k in range(2):
                            last = (tau == 1 and mi == 1 and k == 1)
                            nc.tensor.matmul(
                                gz_cur[:, 128 * mi + off:128 * mi + off + 64],
                                whg_s[k][:, 128 * (mi + 2):128 * (mi + 2) + 128],
                                h_bf[:, 64 * k:64 * k + 64],
                                start=False, stop=last)

                    # x-batch A fills the PE while sigmoid+mul run
                    if j + 1 < NPAIR:
                        if tau == 0:
                            gr_nxt = emit_gx(j + 1, grp, 0, "gr")
                        else:
                            emit_cx(j + 1, 0)

                    gr_v = gr_cur[:, 0:256].rearrange("p (m x) -> p m x", m=2, x=128)
                    gz_v = gz_cur[:, 0:256].rearrange("p (m x) -> p m x", m=2, x=128)
                    r_bf = sp.tile([128, 128], bf16, tag="r", name="r")
                    nc.scalar.activation(
                        r_bf[:].rearrange("p (m b) -> p m b", m=2, b=64),
                        gr_v[:, :, off:off + 64], AF.Sigmoid)
                    zb_bf = sp.tile([128, 128], bf16, tag="zb", name="zb")
                    nc.scalar.activation(
                        zb_bf[:].rearrange("p (m b) -> p m b", m=2, b=64),
                        gz_v[:, :, off:off + 64], AF.Sigmoid, scale=-1.0)

                    rh = sp.tile([128, 128], bf16, tag="rh", name="rh")
                    nc.vector.tensor_mul(rh[:], r_bf[:], h_bf[:])

                    # cand h-part
                    for mi in range(2):
                        for k in range(2):
                            last = (tau == 1 and mi == 1 and k == 1)
                            nc.tensor.matmul(
                                c_cur[:, 128 * mi + off:128 * mi + off + 64],
                                wch_s[k][:, 128 * mi:128 * mi + 128],
                                rh[:, 64 * k:64 * k + 64],
                                start=False, stop=last)

                    # x-batch B in the shadow of tanh + tail
                    if j + 1 < NPAIR:
                        if tau == 0:
                            gz_nxt = emit_gx(j + 1, gzp, 2, "gz")
                        else:
                            c_nxt = emit_cx(j + 1, 1)

                    # z*h = h - zbar*h, off the critical chain
                    s1 = sp.tile([128, 128], bf16, tag="s1", name="s1")
                    nc.vector.tensor_mul(s1[:], zb_bf[:], h_bf[:])
                    hd = sp.tile([128, 128], bf16, tag="hd", name="hd")
                    nc.vector.tensor_sub(hd[:], h_bf[:], s1[:])

                    c_v = c_cur[:, 0:256].rearrange("p (m x) -> p m x", m=2, x=128)
                    c_bf = sp.tile([128, 128], bf16, tag="ct", name="ct")
                    nc.scalar.activation(
                        c_bf[:].rearrange("p (m b) -> p m b", m=2, b=64),
                        c_v[:, :, off:off + 64], AF.Tanh)

                    zbc = sp.tile([128, 128], bf16, tag="zbc", name="zbc")
                    nc.vector.tensor_mul(zbc[:], zb_bf[:], c_bf[:])
                    h_new = pp.tile([128, 128], bf16, tag="hbf", name="hbf", bufs=4)
                    nc.vector.tensor_add(h_new[:], zbc[:], hd[:])
                    h_bf = h_new

                    if tau == 1 and j + 1 < NPAIR:
                        gr_cur, gz_cur, c_cur = gr_nxt, gz_nxt, c_nxt

            # ---- exchange encodings ----
            ag_in = dramp.tile([128, 128], bf16, tag="agin", name="agin")
            ag_out = dramp.tile([NCORES, 128, 128], bf16, tag="agout", name="agout")

            nc.sync.dma_start(ag_in[:], h_bf[:])
            nc.gpsimd.collective_compute(
                "AllGather", mybir.AluOpType.bypass,
                replica_groups=[list(range(NCORES))],
                ins=[ag_in.opt()], outs=[ag_out.opt()])

            # classifier weights + gelu ACT table load overlap the collective
            w1q_s = [pp.tile([128, D_HID], bf16, tag=f"w1q{k}", name=f"w1q{k}") for k in range(2)]
            nc.sync.dma_start(w1q_s[0][:], w1q[0:128, :])
            nc.sync.dma_start(w1q_s[1][:], w1q[128:256, :])
            w1r_s = [pp.tile([128, D_HID], bf16, tag=f"w1r{k}", name=f"w1r{k}") for k in range(2)]
            nc.gpsimd.dma_start(w1r_s[0][:], w1r[0:128, :])
            nc.gpsimd.dma_start(w1r_s[1][:], w1r[128:256, :])
            b1_s = pp.tile([128, 2], f32, tag="b1", name="b1")
            nc.scalar.dma_start(b1_s[:], b1.rearrange("(m p) -> p m", p=128))
            w2_s = [pp.tile([128, D_OUT], bf16, tag=f"w2{k}", name=f"w2{k}") for k in range(2)]
            nc.scalar.dma_start(w2_s[0][:], w2[0:128, :])
            nc.scalar.dma_start(w2_s[1][:], w2[128:256, :])
            nc.sync.dma_start(lhs4[0:1, :], wdt[:])
            nc.sync.dma_start(lhs4[2:3, :], wdt[:])
            nc.gpsimd.dma_start(rhs4[0:4, :], rhsb[:])

            gelu_af = mybir.ActivationFunctionType.Tanh if sim_gelu \
                else mybir.ActivationFunctionType.Gelu_apprx_tanh
            warm_out2 = pp.tile([1, 2], f32, tag="warmo2", name="warmo2")
            nc.scalar.activation(warm_out2[:], warm_in[:], gelu_af)

            # per-core q slice: rows [32*core, 32*core+32) live on gathered
            # block core//2, batch-half core%2 -> one register-offset DMA,
            # issued first since the q1->lhs4 staging chain hangs off it
            qloc = pp.tile([128, 2 * IBLK], bf16, tag="qloc", name="qloc")  # [p, c*32+i]
            pid = nc.sync.partition_id()
            agv = ag_out.rearrange("n p (c h b) -> n p c h b", c=2, h=2, b=32)
            nc.sync.dma_start(
                qloc[:].rearrange("p (c b) -> p c b", c=2, b=32),
                agv[bass.ds(pid >> 1, 1), :, :, bass.ds(pid & 1, 1), :])

            # rT2: whole gathered r blocks; col = 128*kshard + 64*chalf + b
            rT2 = pp.tile([128, 512], bf16, tag="rT2", name="rT2")
            qeng = [nc.gpsimd, nc.scalar, nc.sync, nc.gpsimd]
            for k in range(NSH):
                qeng[k].dma_start(rT2[:, 128 * k:128 * k + 128], ag_out[4 + k])
            # per-hidden-half view with j = 64*kshard + b ordering
            rT_v = rT2[:].rearrange("p (k c b) -> p c k b", k=4, c=2, b=64)

            # ---- classifier ----
            with (
                tc.tile_pool(name="spsum", bufs=1, space="PSUM") as sps,
                tc.tile_pool(name="hpsum", bufs=2, space="PSUM") as hps,
                tc.tile_pool(name="lpsum", bufs=1, space="PSUM") as lps,
                tc.tile_pool(name="cls", bufs=3) as cpool,
            ):
                # PE warm-up: the HAM clock-gate leaves the array at 1.2 GHz
                # after the long collective idle; junk matmuls spanning the
                # preamble restore 2.4 GHz before the main classifier stream
                def warm(n):
                    for _ in range(n):
                        jnk = hps.tile([128, 4 * B], f32, tag="hps", name="hps")
                        nc.tensor.matmul(jnk[:, 0:512], whg_s[0][:, 0:128],
                                         rT2[:], start=True, stop=True)
                    return jnk

                # R1T + b1 first: only needs rT2, overlaps the qloc chain
                r1tb = pp.tile([128, 2 * B], f32, tag="r1tb", name="r1tb")
                for m in range(2):
                    ps3 = sps.tile([128, B], f32, tag="sps", name="sps")
                    for k in range(2):
                        nc.tensor.matmul(ps3[:],
                                         w1r_s[k][:, 128 * m:128 * m + 128],
                                         rT_v[:, k],
                                         start=(k == 0), stop=(k == 1))
                    nc.scalar.activation(r1tb[:, 256 * m:256 * m + 256], ps3[:],
                                         AF.Identity, bias=b1_s[:, m:m + 1])

                # Q1 rows for my i's: [32, 256] bf16
                ps = sps.tile([IBLK, D_HID], f32, tag="sps", name="sps")
                for c in range(2):
                    nc.tensor.matmul(ps[:], qloc[:, 32 * c:32 * c + 32],
                                     w1q_s[c][:], start=(c == 0), stop=(c == 1))
                q1 = pp.tile([IBLK, D_HID], bf16, tag="q1", name="q1")
                nc.scalar.activation(q1[:], ps[:], AF.Copy, bias=0.0)
                nc.sync.dma_start(lhs4[1:2, :], q1[0:16, :])
                nc.gpsimd.dma_start(lhs4[3:4, :], q1[16:32, :])

                # dist rows for my i's: [32, 256] bf16
                ps2 = sps.tile([IBLK, B], f32, tag="sps", name="sps")
                for c in range(2):
                    nc.tensor.matmul(ps2[:], qloc[:, 32 * c:32 * c + 32],
                                     rT_v[:, c], start=(c == 0), stop=(c == 1))
                dist = pp.tile([IBLK, B], bf16, tag="dist", name="dist")
                nc.scalar.activation(dist[:], ps2[:], AF.Copy, bias=0.0)
                nc.gpsimd.dma_start(
                    rhs4[0:1, :].rearrange("o (p ii j) -> o p ii j",
                                           p=IBLK // 2, ii=2, j=B)[:, :, 0, :],
                    dist[0:16, :])
                nc.sync.dma_start(
                    rhs4[2:3, :].rearrange("o (p ii j) -> o p ii j",
                                           p=IBLK // 2, ii=2, j=B)[:, :, 1, :],
                    dist[16:32, :])

                r1tb2 = pp.tile([128, 4 * B], f32, tag="r1tb2", name="r1tb2")
                r2v = r1tb2[:].rearrange("p (m ii j) -> p m ii j", m=2, ii=2,
                                         j=B)
                for ii in range(2):
                    nc.vector.tensor_copy(
                        r2v[:, :, ii, :],
                        r1tb[:].rearrange("p (m j) -> p m j", m=2, j=B))

                jnk = warm(12)
                warm_sb = pp.tile([128, 4], f32, tag="warm3", name="warm3")
                nc.vector.tensor_copy(warm_sb[:], jnk[:, 0:4])
                warm_dram = dramp.tile([128, 4], f32, tag="warmd", name="warmd")
                nc.scalar.dma_start(warm_dram[:], warm_sb[:])

                out_sb = pp.tile([D_OUT, IBLK * B], f32, tag="outsb", name="outsb")
                # pair-rows processed two at a time so gelu and the output
                # copy amortize their fixed cost over 1024 columns; the output
                # copy alternates Scalar/Vector to balance engine load (b2 is
                # added on the host)
                # r1tb4: r1 operand duplicated over (m, sub) for 1024-wide adds
                r1tb4 = pp.tile([128, 8 * B], f32, tag="r1tb4", name="r1tb4")
                for m in range(2):
                    for sub in range(2):
                        nc.vector.tensor_copy(
                            r1tb4[:, 1024 * m + 512 * sub:
                                  1024 * m + 512 * sub + 512],
                            r1tb2[:, 512 * m:512 * m + 512])

                def emit_w2(prp, h1):
                    """Second layer + output copy for pair-row-pair prp."""
                    l_ps = lps.tile([D_OUT, 4 * B], f32, tag="lps", name="lps")
                    for sub in range(2):
                        for k in range(2):
                            nc.tensor.matmul(
                                l_ps[:, 512 * sub:512 * sub + 512], w2_s[k][:],
                                h1[:, 1024 * k + 512 * sub:
                                   1024 * k + 512 * sub + 512],
                                start=(k == 0), stop=(k == 1))
                    oseg = out_sb[:, 1024 * prp:1024 * prp + 1024]
                    # 6:2 scalar:vector split equalizes engine load (vector
                    # already carries the two 1024-wide adds per iteration)
                    if prp % 4 == 3:
                        nc.vector.tensor_copy(oseg, l_ps[:])
                    else:
                        nc.scalar.activation(oseg, l_ps[:], AF.Copy, bias=0.0)
                    [nc.sync, nc.gpsimd][prp % 2].dma_start(
                        out[:, 1024 * prp:1024 * prp + 1024], oseg)

                # software-pipelined: W2(prp-1) is emitted after the h1 MMs of
                # prp so the in-order PE never stalls on gelu(prp)
                prev = None
                for prp in range(IBLK // 4):
                    pr0 = 2 * prp
                    # h1 block layout: col = 1024*m + 512*sub + 256*ii + j
                    h1p4 = cpool.tile([128, 8 * B], bf16, tag="h1p", name="h1p")
                    for m in range(2):
                        h_ps = hps.tile([128, 4 * B], f32, tag="hps",
                                        name="hps")
                        for sub in range(2):
                            pr = pr0 + sub
                            nc.tensor.matmul(
                                h_ps[:, 512 * sub:512 * sub + 512],
                                lhs4[0:128, D_HID * pr + 128 * m:
                                     D_HID * pr + 128 * m + 128],
                                rhs4[0:128, 2 * B * pr:2 * B * pr + 2 * B],
                                start=True, stop=True)
                        nc.vector.tensor_add(
                            h1p4[:, 1024 * m:1024 * m + 1024], h_ps[:],
                            r1tb4[:, 1024 * m:1024 * m + 1024])
                    h1 = cpool.tile([128, 8 * B], bf16, tag="h1", name="h1")
                    nc.scalar.activation(h1[:], h1p4[:], gelu_af)
                    if prev is not None:
                        emit_w2(prev[0], prev[1])
                    prev = (prp, h1)
                emit_w2(prev[0], prev[1])

    nc.compile()
    return nc


def _rhs_base():
    """[4, IBLK*B] pattern: per 512-col pair-block rows are
    [0,0],[ones,0],[0,0],[0,ones] - dist blocks get DMA'd in on device."""
    r = np.zeros((4, IBLK * B), dtype=BF16)
    v = r.reshape(4, IBLK // 2, 2, B)
    v[1, :, 0, :] = 1.0
    v[3, :, 1, :] = 1.0
    return r


def _prep_inputs(inputs):
    """Host-side prep: embed+transpose sequences, split weights, per-core maps."""
    emb = inputs["embeddings"]
    in_maps = []
    f32 = np.float32

    # classifier tensors (identical on all cores)
    W1, b1, W2, b2 = (inputs["W1"], inputs["b1"], inputs["W2"], inputs["b2"])
    common = {
        "w1q": np.ascontiguousarray(W1[:H]).astype(BF16),
        "w1r": np.ascontiguousarray(W1[H + 1:]).astype(BF16),
        "wdt": np.tile(np.ascontiguousarray(W1[H:H + 1]).astype(BF16),
                       (1, IBLK // 2)),
        "rhsb": _rhs_base(),
        "b1": b1.astype(f32),
        "w2": W2.astype(BF16),
    }

    ones_row = np.ones((1, BT), f32)
    for core in range(NCORES):
        enc = core // NSH
        s = core % NSH
        if enc == 0:
            seqs, lens = inputs["input_queries"], inputs["query_lengths"]
            Wg, bgv, Wc, bcv = (inputs["Wg_q"], inputs["bg_q"],
                                inputs["Wc_q"], inputs["bc_q"])
        else:
            seqs, lens = inputs["input_replies"], inputs["reply_lengths"]
            Wg, bgv, Wc, bcv = (inputs["Wg_r"], inputs["bg_r"],
                                inputs["Wc_r"], inputs["bc_r"])
        rows = slice(BSH * s, BSH * s + BSH)
        xe = emb[seqs[rows]]                       # [64, 40, 256]
        xT = np.transpose(xe, (2, 1, 0)).reshape(E, BT)  # col = t*64+b
        lmask = (np.arange(T)[:, None] >= lens[rows][None, :]) \
            .astype(f32).reshape(1, BT)
        xembT = np.concatenate([xT, lmask, ones_row], axis=0).astype(BF16)

        mask_row = np.concatenate([np.zeros(H, f32), np.full(H, 30.0, f32)])
        wxgm = np.stack([mask_row, bgv.astype(f32)]).astype(BF16)
        wxcm = np.stack([np.zeros(H, f32), bcv.astype(f32)]).astype(BF16)

        m = {
            "xembT": xembT,
            "whg": np.ascontiguousarray(Wg[E:]).astype(BF16),
            "wxg": np.ascontiguousarray(Wg[:E]).astype(BF16),
            "wxgm": wxgm,
            "wch": np.ascontiguousarray(Wc[E:]).astype(BF16),
            "wxc": np.ascontiguousarray(Wc[:E]).astype(BF16),
            "wxcm": wxcm,
        }
        m.update(common)
        in_maps.append(m)
    return in_maps


def run_cores(in_maps, trace=False):
    from concourse.bass_utils import run_bass_kernel_spmd
    from concourse.bass_interp import get_hw_module

    if "nc" not in _cache:
        _cache["nc"] = _build()
    nc = _cache["nc"]
    old = nc.m
    nc.m = _cache.setdefault("hwm", get_hw_module(nc.m))
    try:
        res = run_bass_kernel_spmd(nc, in_maps, core_ids=list(range(NCORES)),
                                   trace=trace)
    finally:
        nc.m = old
    return res


def kernel(**inputs):
    in_maps = _prep_inputs(inputs)
    res = run_cores(in_maps)
    logits = np.zeros((B, B, 2), np.float32)
    for core in range(NCORES):
        o = res.results[core]["out"]               # [2, 32*256]
        # pair layout: col = 512*pr + 256*ii + j, local row = 16*ii + pr
        logits[IBLK * core:IBLK * core + IBLK] = \
            o.reshape(2, 16, 2, B).transpose(2, 1, 3, 0).reshape(IBLK, B, 2)
    logits += inputs["b2"].astype(np.float32)[None, None, :]
    pos = logits[np.arange(B), np.arange(B)]
    qi, ri = np.nonzero(~np.eye(B, dtype=bool))
    neg = logits[qi, ri]
    return np.concatenate([pos, neg], axis=0).astype(np.float32)


if __name__ == "__main__":
    _build()
    print("build OK")


# revision 31
# speedup vs baseline: 1.2853x; 1.0391x over previous
"""Trainium2 Bass kernel for nn_BestModel5 (dual-GRU encoder + BxB pair classifier).

Sharding (8 cores): cores 0-3 query-GRU batch shards of 64; cores 4-7 reply-GRU.
Classifier sharded 8-way over the 256 query rows (32 i-rows/core).
Embedding gather + layout prep on host; all matmuls bf16 on PE, f32 PSUM.

GRU step pipeline: x-projections (incl. length-mask row and a ones-row that
carries the gate biases, both zero-padded to K=128 - a short stationary
operand pins LDWEIGHTS to row-group q0 and serializes the PE) are
matmul-accumulated straight into PSUM two steps per bank, so the recurrent
chain is just
  h-MMs -> sigmoid(PSUM) -> r*h -> cand-MMs -> tanh(PSUM) -> zbar*c -> + z*h
with all elementwise ops in bf16 (2x DVE mode) and z*h = h - zbar*h built
off-chain from sigmoid(-zpre).
"""

import numpy as np
import ml_dtypes

BF16 = ml_dtypes.bfloat16

V, E, H, B, T = 100000, 256, 256, 256, 40
D_HID, D_OUT = 256, 2
NCORES = 8
BSH = 64            # batch rows per GRU shard
NSH = 4             # GRU batch shards per encoder
BT = BSH * T        # 2560 columns of xembT per core
IBLK = B // NCORES  # 32 classifier i-rows per core
NPAIR = T // 2      # PSUM step-pairs

_cache = {}


def _build(sim_gelu=False):
    """Build + compile the SPMD Bass program once."""
    import concourse.bacc as bacc
    import concourse.bass as bass
    import concourse.tile as tile
    import concourse.mybir as mybir

    f32 = mybir.dt.float32
    bf16 = mybir.dt.bfloat16
    AF = mybir.ActivationFunctionType

    nc = bacc.Bacc("TRN2", target_bir_lowering=False, debug=False, num_devices=NCORES)

    def din(name, shape, dt):
        return nc.dram_tensor(name, shape, dt, kind="ExternalInput").ap()

    # per-core inputs (content differs per core; shapes identical)
    xembT = din("xembT", [E + 2, BT], bf16)      # rows 0-255 emb, 256 mask, 257 ones
    whg = din("whg", [H, 2 * H], bf16)           # Wg[E:E+H, :]
    wxg = din("wxg", [E, 2 * H], bf16)           # Wg[:E, :]
    wxgm = din("wxgm", [2, 2 * H], bf16)         # row0 = [0|30] mask row, row1 = bg
    wch = din("wch", [H, H], bf16)               # Wc[E:E+H, :]
    wxc = din("wxc", [E, H], bf16)               # Wc[:E, :]
    wxcm = din("wxcm", [2, H], bf16)             # row0 = 0, row1 = bc
    w1q = din("w1q", [H, D_HID], bf16)           # W1[:256]
    w1r = din("w1r", [H, D_HID], bf16)           # W1[257:513]
    wdt = din("wdt", [1, IBLK // 2 * D_HID], bf16)  # W1[256] tiled 16x
    rhsb = din("rhsb", [4, IBLK * B], bf16)      # [0;ones|0;0|0;0;ones] pattern
    b1 = din("b1", [D_HID], f32)
    w2 = din("w2", [D_HID, D_OUT], bf16)

    out = nc.dram_tensor("out", [D_OUT, IBLK * B], f32, kind="ExternalOutput").ap()

    with tile.TileContext(nc) as tc:
        with (
            tc.tile_pool(name="persist", bufs=1) as pp,
            tc.tile_pool(name="dram", bufs=1, space="DRAM") as dramp,
        ):
            # ---- loads: chunk 0 + x-weights first so the recurrence can
            # start immediately; h-weights next; classifier weights deferred ----
            xT = [pp.tile([128, BT], bf16, tag=f"xT{k}", name=f"xT{k}") for k in range(2)]
            xTm = pp.tile([128, BT], bf16, tag="xTm", name="xTm")
            nc.vector.memset(xTm[:, 0:256], 0.0)

            cs0 = slice(0, 256)
            nc.sync.dma_start(xT[0][:, cs0], xembT[0:128, cs0])
            nc.gpsimd.dma_start(xT[1][:, cs0], xembT[128:256, cs0])
            nc.scalar.dma_start(xTm[0:2, cs0], xembT[256:258, cs0])

            wxgm_s = pp.tile([128, 2 * H], bf16, tag="wxgm", name="wxgm")
            nc.vector.memset(wxgm_s[:], 0.0)
            wxcm_s = pp.tile([128, H], bf16, tag="wxcm", name="wxcm")
            nc.vector.memset(wxcm_s[:], 0.0)
            nc.vector.memset(xTm[:, 256:BT], 0.0)

            wxg_s = [pp.tile([128, 2 * H], bf16, tag=f"wxg{k}", name=f"wxg{k}") for k in range(2)]
            nc.sync.dma_start(wxg_s[0][:], wxg[0:128, :])
            nc.gpsimd.dma_start(wxg_s[1][:], wxg[128:256, :])
            nc.scalar.dma_start(wxgm_s[0:2, :], wxgm[:])
            wxc_s = [pp.tile([128, H], bf16, tag=f"wxc{k}", name=f"wxc{k}") for k in range(2)]
            nc.sync.dma_start(wxc_s[0][:], wxc[0:128, :])
            nc.gpsimd.dma_start(wxc_s[1][:], wxc[128:256, :])
            nc.scalar.dma_start(wxcm_s[0:2, :], wxcm[:])

            whg_s = [pp.tile([128, 2 * H], bf16, tag=f"whg{k}", name=f"whg{k}") for k in range(2)]
            nc.sync.dma_start(whg_s[0][:], whg[0:128, :])
            nc.gpsimd.dma_start(whg_s[1][:], whg[128:256, :])
            wch_s = [pp.tile([128, H], bf16, tag=f"wch{k}", name=f"wch{k}") for k in range(2)]
            nc.sync.dma_start(wch_s[0][:], wch[0:128, :])
            nc.gpsimd.dma_start(wch_s[1][:], wch[128:256, :])

            # preload sigmoid/tanh ACT table during DMA wait
            warm_in = pp.tile([1, 2], f32, tag="warmi", name="warmi")
            nc.vector.memset(warm_in[:], 0.0)
            warm_out = pp.tile([1, 2], f32, tag="warmo", name="warmo")
            nc.scalar.activation(warm_out[:], warm_in[:], AF.Sigmoid)

            # remaining xembT in 256-col (2-pair) chunks, in step order
            dq = [nc.sync, nc.gpsimd, nc.scalar]
            for J in range(1, NPAIR // 2):
                cs = slice(256 * J, 256 * J + 256)
                dq[J % 3].dma_start(xT[0][:, cs], xembT[0:128, cs])
                dq[(J + 1) % 3].dma_start(xT[1][:, cs], xembT[128:256, cs])
                dq[(J + 2) % 3].dma_start(xTm[0:2, cs], xembT[256:258, cs])

            # classifier static operands (pad rows 4-127 with zeros so the
            # stationary operand spans all PE row groups)
            lhs4 = pp.tile([128, IBLK // 2 * D_HID], bf16, tag="lhs4", name="lhs4")
            nc.vector.memset(lhs4[:], 0.0)
            rhs4 = pp.tile([128, IBLK * B], bf16, tag="rhs4", name="rhs4")
            nc.vector.memset(rhs4[:], 0.0)

            # ---- GRU recurrence ----
            with (
                tc.tile_pool(name="grpsum", bufs=2, space="PSUM") as grp,
                tc.tile_pool(name="gzpsum", bufs=2, space="PSUM") as gzp,
                tc.tile_pool(name="cpsum", bufs=3, space="PSUM") as cp,
                tc.tile_pool(name="step", bufs=6) as sp,
            ):
                def emit_gx(j, pool, mbase, tag):
                    """Gates x-part (2 m-blocks) for pair j; cols 128*m+64*tau+b."""
                    g = pool.tile([128, 512], f32, tag=tag, name=f"{tag}{j}")
                    cs = slice(128 * j, 128 * j + 128)
                    for mi in range(2):
                        m = mbase + mi
                        ms = slice(128 * m, 128 * m + 128)
                        os = slice(128 * mi, 128 * mi + 128)
                        nc.tensor.matmul(g[:, os], wxg_s[0][:, ms], xT[0][:, cs],
                                         start=(mi == 0), stop=False)
                        nc.tensor.matmul(g[:, os], wxg_s[1][:, ms], xT[1][:, cs],
                                         start=False, stop=False)
                        nc.tensor.matmul(g[:, os], wxgm_s[:, ms], xTm[:, cs],
                                         start=False, stop=False)
                    return g

                def emit_cx(j, mi):
                    """Cand x-part m-block mi for pair j (new tile when mi=0)."""
                    if mi == 0:
                        c = cp.tile([128, 512], f32, tag="c", name=f"c{j}")
                        emit_cx.cur = c
                    c = emit_cx.cur
                    cs = slice(128 * j, 128 * j + 128)
                    ms = slice(128 * mi, 128 * mi + 128)
                    nc.tensor.matmul(c[:, ms], wxc_s[0][:, ms], xT[0][:, cs],
                                     start=(mi == 0), stop=False)
                    nc.tensor.matmul(c[:, ms], wxc_s[1][:, ms], xT[1][:, cs],
                                     start=False, stop=False)
                    nc.tensor.matmul(c[:, ms], wxcm_s[:, ms], xTm[:, cs],
                                     start=False, stop=False)
                    return c

                h_bf = pp.tile([128, 128], bf16, tag="hbf", name="hbf", bufs=4)
                nc.vector.memset(h_bf[:], 0.0)

                gr_cur = emit_gx(0, grp, 0, "gr")
                gz_cur = emit_gx(0, gzp, 2, "gz")
                emit_cx(0, 0)
                c_cur = emit_cx(0, 1)
                gr_nxt = gz_nxt = c_nxt = None

                for t in range(T):
                    j, tau = t // 2, t % 2
                    off = 64 * tau

                    # gates h-part: r then z
                    for mi in range(2):
                        for k in range(2):
                            last = (tau == 1 and mi == 1 and k == 1)
                            nc.tensor.matmul(
                                gr_cur[:, 128 * mi + off:128 * mi + off + 64],
                                whg_s[k][:, 128 * mi:128 * mi + 128],
                                h_bf[:, 64 * k:64 * k + 64],
                                start=False, stop=last)
                    for mi in range(2):
                        for k in range(2):
                            last = (tau == 1 and mi == 1 and k == 1)
                            nc.tensor.matmul(
                                gz_cur[:, 128 * mi + off:128 * mi + off + 64],
                                whg_s[k][:, 128 * (mi + 2):128 * (mi + 2) + 128],
                                h_bf[:, 64 * k:64 * k + 64],
                                start=False, stop=last)

                    # x-batch A fills the PE while sigmoid+mul run
                    if j + 1 < NPAIR:
                        if tau == 0:
                            gr_nxt = emit_gx(j + 1, grp, 0, "gr")
                        else:
                            emit_cx(j + 1, 0)

                    gr_v = gr_cur[:, 0:256].rearrange("p (m x) -> p m x", m=2, x=128)
                    gz_v = gz_cur[:, 0:256].rearrange("p (m x) -> p m x", m=2, x=128)
                    r_bf = sp.tile([128, 128], bf16, tag="r", name="r")
                    nc.scalar.activation(
                        r_bf[:].rearrange("p (m b) -> p m b", m=2, b=64),
                        gr_v[:, :, off:off + 64], AF.Sigmoid)
                    zb_bf = sp.tile([128, 128], bf16, tag="zb", name="zb")
                    nc.scalar.activation(
                        zb_bf[:].rearrange("p (m b) -> p m b", m=2, b=64),
                        gz_v[:, :, off:off + 64], AF.Sigmoid, scale=-1.0)

                    rh = sp.tile([128, 128], bf16, tag="rh", name="rh")
                    nc.vector.tensor_mul(rh[:], r_bf[:], h_bf[:])

                    # cand h-part
                    for mi in range(2):
                        for k in range(2):
                            last = (tau == 1 and mi == 1 and k == 1)
                            nc.tensor.matmul(
                                c_cur[:, 128 * mi + off:128 * mi + off + 64],
                                wch_s[k][:, 128 * mi:128 * mi + 128],
                                rh[:, 64 * k:64 * k + 64],
                                start=False, stop=last)

                    # x-batch B in the shadow of tanh + tail
                    if j + 1 < NPAIR:
                        if tau == 0:
                            gz_nxt = emit_gx(j + 1, gzp, 2, "gz")
                        else:
                            c_nxt = emit_cx(j + 1, 1)

                    # z*h = h - zbar*h, off the critical chain
                    s1 = sp.tile([128, 128], bf16, tag="s1", name="s1")
                    nc.vector.tensor_mul(s1[:], zb_bf[:], h_bf[:])
                    hd = sp.tile([128, 128], bf16, tag="hd", name="hd")
                    nc.vector.tensor_sub(hd[:], h_bf[:], s1[:])

                    c_v = c_cur[:, 0:256].rearrange("p (m x) -> p m x", m=2, x=128)
                    c_bf = sp.tile([128, 128], bf16, tag="ct", name="ct")
                    nc.scalar.activation(
                        c_bf[:].rearrange("p (m b) -> p m b", m=2, b=64),
                        c_v[:, :, off:off + 64], AF.Tanh)

                    zbc = sp.tile([128, 128], bf16, tag="zbc", name="zbc")
                    nc.vector.tensor_mul(zbc[:], zb_bf[:], c_bf[:])
                    h_new = pp.tile([128, 128], bf16, tag="hbf", name="hbf", bufs=4)
                    nc.vector.tensor_add(h_new[:], zbc[:], hd[:])
                    h_bf = h_new

                    if tau == 1 and j + 1 < NPAIR:
                        gr_cur, gz_cur, c_cur = gr_nxt, gz_nxt, c_nxt

            # ---- exchange encodings ----
            ag_in = dramp.tile([128, 128], bf16, tag="agin", name="agin")
            ag_out = dramp.tile([NCORES, 128, 128], bf16, tag="agout", name="agout")

            nc.sync.dma_start(ag_in[:], h_bf[:])
            nc.gpsimd.collective_compute(
                "AllGather", mybir.AluOpType.bypass,
                replica_groups=[list(range(NCORES))],
                ins=[ag_in.opt()], outs=[ag_out.opt()])

            # classifier weights + gelu ACT table load overlap the collective
            w1q_s = [pp.tile([128, D_HID], bf16, tag=f"w1q{k}", name=f"w1q{k}") for k in range(2)]
            nc.sync.dma_start(w1q_s[0][:], w1q[0:128, :])
            nc.sync.dma_start(w1q_s[1][:], w1q[128:256, :])
            w1r_s = [pp.tile([128, D_HID], bf16, tag=f"w1r{k}", name=f"w1r{k}") for k in range(2)]
            nc.gpsimd.dma_start(w1r_s[0][:], w1r[0:128, :])
            nc.gpsimd.dma_start(w1r_s[1][:], w1r[128:256, :])
            b1_s = pp.tile([128, 2], f32, tag="b1", name="b1")
            nc.scalar.dma_start(b1_s[:], b1.rearrange("(m p) -> p m", p=128))
            w2_s = [pp.tile([128, D_OUT], bf16, tag=f"w2{k}", name=f"w2{k}") for k in range(2)]
            nc.scalar.dma_start(w2_s[0][:], w2[0:128, :])
            nc.scalar.dma_start(w2_s[1][:], w2[128:256, :])
            nc.sync.dma_start(lhs4[0:1, :], wdt[:])
            nc.sync.dma_start(lhs4[2:3, :], wdt[:])
            nc.gpsimd.dma_start(rhs4[0:4, :], rhsb[:])

            gelu_af = mybir.ActivationFunctionType.Tanh if sim_gelu \
                else mybir.ActivationFunctionType.Gelu_apprx_tanh
            warm_out2 = pp.tile([1, 2], f32, tag="warmo2", name="warmo2")
            nc.scalar.activation(warm_out2[:], warm_in[:], gelu_af)

            # per-core q slice: rows [32*core, 32*core+32) live on gathered
            # block core//2, batch-half core%2 -> one register-offset DMA,
            # issued first since the q1->lhs4 staging chain hangs off it
            qloc = pp.tile([128, 2 * IBLK], bf16, tag="qloc", name="qloc")  # [p, c*32+i]
            pid = nc.sync.partition_id()
            agv = ag_out.rearrange("n p (c h b) -> n p c h b", c=2, h=2, b=32)
            nc.sync.dma_start(
                qloc[:].rearrange("p (c b) -> p c b", c=2, b=32),
                agv[bass.ds(pid >> 1, 1), :, :, bass.ds(pid & 1, 1), :])

            # rT2: whole gathered r blocks; col = 128*kshard + 64*chalf + b
            rT2 = pp.tile([128, 512], bf16, tag="rT2", name="rT2")
            qeng = [nc.gpsimd, nc.scalar, nc.sync]
            for k in range(3):
                qeng[k].dma_start(rT2[:, 128 * k:128 * k + 128], ag_out[4 + k])
            nc.gpsimd.dma_start(rT2[:, 384:448], ag_out[7, :, 0:64])
            nc.scalar.dma_start(rT2[:, 448:512], ag_out[7, :, 64:128])
            # per-hidden-half view with j = 64*kshard + b ordering
            rT_v = rT2[:].rearrange("p (k c b) -> p c k b", k=4, c=2, b=64)

            # ---- classifier ----
            with (
                tc.tile_pool(name="spsum", bufs=1, space="PSUM") as sps,
                tc.tile_pool(name="hpsum", bufs=2, space="PSUM") as hps,
                tc.tile_pool(name="lpsum", bufs=1, space="PSUM") as lps,
                tc.tile_pool(name="cls", bufs=3) as cpool,
            ):
                # PE warm-up: the HAM clock-gate leaves the array at 1.2 GHz
                # after the long collective idle; junk matmuls spanning the
                # preamble restore 2.4 GHz before the main classifier stream
                def warm(n):
                    for _ in range(n):
                        jnk = hps.tile([128, 4 * B], f32, tag="hps", name="hps")
                        nc.tensor.matmul(jnk[:, 0:512], whg_s[0][:, 0:128],
                                         rT2[:], start=True, stop=True)
                    return jnk

                # R1T + b1 first: only needs rT2, overlaps the qloc chain
                r1tb = pp.tile([128, 2 * B], f32, tag="r1tb", name="r1tb")
                for m in range(2):
                    ps3 = sps.tile([128, B], f32, tag="sps", name="sps")
                    for k in range(2):
                        nc.tensor.matmul(ps3[:],
                                         w1r_s[k][:, 128 * m:128 * m + 128],
                                         rT_v[:, k],
                                         start=(k == 0), stop=(k == 1))
                    nc.scalar.activation(r1tb[:, 256 * m:256 * m + 256], ps3[:],
                                         AF.Identity, bias=b1_s[:, m:m + 1])

                # Q1 rows for my i's: [32, 256] bf16
                ps = sps.tile([IBLK, D_HID], f32, tag="sps", name="sps")
                for c in range(2):
                    nc.tensor.matmul(ps[:], qloc[:, 32 * c:32 * c + 32],
                                     w1q_s[c][:], start=(c == 0), stop=(c == 1))
                q1 = pp.tile([IBLK, D_HID], bf16, tag="q1", name="q1")
                nc.scalar.activation(q1[:], ps[:], AF.Copy, bias=0.0)
                nc.sync.dma_start(lhs4[1:2, :], q1[0:16, :])
                nc.gpsimd.dma_start(lhs4[3:4, :], q1[16:32, :])

                # dist rows for my i's: [32, 256] bf16
                ps2 = sps.tile([IBLK, B], f32, tag="sps", name="sps")
                for c in range(2):
                    nc.tensor.matmul(ps2[:], qloc[:, 32 * c:32 * c + 32],
                                     rT_v[:, c], start=(c == 0), stop=(c == 1))
                dist = pp.tile([IBLK, B], bf16, tag="dist", name="dist")
                nc.scalar.activation(dist[:], ps2[:], AF.Copy, bias=0.0)
                nc.gpsimd.dma_start(
                    rhs4[0:1, :].rearrange("o (p ii j) -> o p ii j",
                                           p=IBLK // 2, ii=2, j=B)[:, :, 0, :],
                    dist[0:16, :])
                nc.sync.dma_start(
                    rhs4[2:3, :].rearrange("o (p ii j) -> o p ii j",
                                           p=IBLK // 2, ii=2, j=B)[:, :, 1, :],
                    dist[16:32, :])

                r1tb2 = pp.tile([128, 4 * B], f32, tag="r1tb2", name="r1tb2")
                r2v = r1tb2[:].rearrange("p (m ii j) -> p m ii j", m=2, ii=2,
                                         j=B)
                for ii in range(2):
                    nc.vector.tensor_copy(
                        r2v[:, :, ii, :],
                        r1tb[:].rearrange("p (m j) -> p m j", m=2, j=B))

                jnk = warm(12)
                warm_sb = pp.tile([128, 4], f32, tag="warm3", name="warm3")
                nc.vector.tensor_copy(warm_sb[:], jnk[:, 0:4])
                warm_dram = dramp.tile([128, 4], f32, tag="warmd", name="warmd")
                nc.scalar.dma_start(warm_dram[:], warm_sb[:])

                out_sb = pp.tile([D_OUT, IBLK * B], f32, tag="outsb", name="outsb")
                # pair-rows processed two at a time so gelu and the output
                # copy amortize their fixed cost over 1024 columns; the output
                # copy alternates Scalar/Vector to balance engine load (b2 is
                # added on the host)
                # r1tb4: r1 operand duplicated over (m, sub) for 1024-wide adds
                r1tb4 = pp.tile([128, 8 * B], f32, tag="r1tb4", name="r1tb4")
                for m in range(2):
                    for sub in range(2):
                        nc.vector.tensor_copy(
                            r1tb4[:, 1024 * m + 512 * sub:
                                  1024 * m + 512 * sub + 512],
                            r1tb2[:, 512 * m:512 * m + 512])

                def emit_w2(prp, h1):
                    """Second layer + output copy for pair-row-pair prp."""
                    l_ps = lps.tile([D_OUT, 4 * B], f32, tag="lps", name="lps")
                    for sub in range(2):
                        for k in range(2):
                            nc.tensor.matmul(
                                l_ps[:, 512 * sub:512 * sub + 512], w2_s[k][:],
                                h1[:, 1024 * k + 512 * sub:
                                   1024 * k + 512 * sub + 512],
                                start=(k == 0), stop=(k == 1))
                    oseg = out_sb[:, 1024 * prp:1024 * prp + 1024]
                    # 6:2 scalar:vector split equalizes engine load (vector
                    # already carries the two 1024-wide adds per iteration)
                    if prp % 4 == 3:
                        nc.vector.tensor_copy(oseg, l_ps[:])
                    else:
                        nc.scalar.activation(oseg, l_ps[:], AF.Copy, bias=0.0)
                    [nc.sync, nc.gpsimd][prp % 2].dma_start(
                        out[:, 1024 * prp:1024 * prp + 1024], oseg)

                # software-pipelined: W2(prp-1) is emitted after the h1 MMs of
                # prp so the in-order PE never stalls on gelu(prp)
                prev = None
                for prp in range(IBLK // 4):
                    pr0 = 2 * prp
                    # h1 block layout: col = 1024*m + 512*sub + 256*ii + j
                    h1p4 = cpool.tile([128, 8 * B], bf16, tag="h1p", name="h1p")
                    for m in range(2):
                        h_ps = hps.tile([128, 4 * B], f32, tag="hps",
                                        name="hps")
                        for sub in range(2):
                            pr = pr0 + sub
                            nc.tensor.matmul(
                                h_ps[:, 512 * sub:512 * sub + 512],
                                lhs4[0:128, D_HID * pr + 128 * m:
                                     D_HID * pr + 128 * m + 128],
                                rhs4[0:128, 2 * B * pr:2 * B * pr + 2 * B],
                                start=True, stop=True)
                        nc.vector.tensor_add(
                            h1p4[:, 1024 * m:1024 * m + 1024], h_ps[:],
                            r1tb4[:, 1024 * m:1024 * m + 1024])
                    h1 = cpool.tile([128, 8 * B], bf16, tag="h1", name="h1")
                    nc.scalar.activation(h1[:], h1p4[:], gelu_af)
                    if prev is not None:
                        emit_w2(prev[0], prev[1])
                    prev = (prp, h1)
                l_ps = lps.tile([D_OUT, 4 * B], f32, tag="lps", name="lps")
                for sub in range(2):
                    for k in range(2):
                        nc.tensor.matmul(
                            l_ps[:, 512 * sub:512 * sub + 512], w2_s[k][:],
                            prev[1][:, 1024 * k + 512 * sub:
                                    1024 * k + 512 * sub + 512],
                            start=(k == 0), stop=(k == 1))
                lprp = prev[0]
                oseg = out_sb[:, 1024 * lprp:1024 * lprp + 1024]
                nc.scalar.activation(oseg[:, 0:512], l_ps[:, 0:512],
                                     AF.Copy, bias=0.0)
                nc.vector.tensor_copy(oseg[:, 512:1024], l_ps[:, 512:1024])
                nc.sync.dma_start(out[:, 1024 * lprp:1024 * lprp + 512],
                                  oseg[:, 0:512])
                nc.gpsimd.dma_start(
                    out[:, 1024 * lprp + 512:1024 * lprp + 1024],
                    oseg[:, 512:1024])

    nc.compile()
    return nc


def _rhs_base():
    """[4, IBLK*B] pattern: per 512-col pair-block rows are
    [0,0],[ones,0],[0,0],[0,ones] - dist blocks get DMA'd in on device."""
    r = np.zeros((4, IBLK * B), dtype=BF16)
    v = r.reshape(4, IBLK // 2, 2, B)
    v[1, :, 0, :] = 1.0
    v[3, :, 1, :] = 1.0
    return r


def _prep_inputs(inputs):
    """Host-side prep: embed+transpose sequences, split weights, per-core maps."""
    emb = inputs["embeddings"]
    in_maps = []
    f32 = np.float32

    # classifier tensors (identical on all cores)
    W1, b1, W2, b2 = (inputs["W1"], inputs["b1"], inputs["W2"], inputs["b2"])
    common = {
        "w1q": np.ascontiguousarray(W1[:H]).astype(BF16),
        "w1r": np.ascontiguousarray(W1[H + 1:]).astype(BF16),
        "wdt": np.tile(np.ascontiguousarray(W1[H:H + 1]).astype(BF16),
                       (1, IBLK // 2)),
        "rhsb": _rhs_base(),
        "b1": b1.astype(f32),
        "w2": W2.astype(BF16),
    }

    ones_row = np.ones((1, BT), f32)
    for core in range(NCORES):
        enc = core // NSH
        s = core % NSH
        if enc == 0:
            seqs, lens = inputs["input_queries"], inputs["query_lengths"]
            Wg, bgv, Wc, bcv = (inputs["Wg_q"], inputs["bg_q"],
                                inputs["Wc_q"], inputs["bc_q"])
        else:
            seqs, lens = inputs["input_replies"], inputs["reply_lengths"]
            Wg, bgv, Wc, bcv = (inputs["Wg_r"], inputs["bg_r"],
                                inputs["Wc_r"], inputs["bc_r"])
        rows = slice(BSH * s, BSH * s + BSH)
        xe = emb[seqs[rows]]                       # [64, 40, 256]
        xT = np.transpose(xe, (2, 1, 0)).reshape(E, BT)  # col = t*64+b
        lmask = (np.arange(T)[:, None] >= lens[rows][None, :]) \
            .astype(f32).reshape(1, BT)
        xembT = np.concatenate([xT, lmask, ones_row], axis=0).astype(BF16)

        mask_row = np.concatenate([np.zeros(H, f32), np.full(H, 30.0, f32)])
        wxgm = np.stack([mask_row, bgv.astype(f32)]).astype(BF16)
        wxcm = np.stack([np.zeros(H, f32), bcv.astype(f32)]).astype(BF16)

        m = {
            "xembT": xembT,
            "whg": np.ascontiguousarray(Wg[E:]).astype(BF16),
            "wxg": np.ascontiguousarray(Wg[:E]).astype(BF16),
            "wxgm": wxgm,
            "wch": np.ascontiguousarray(Wc[E:]).astype(BF16),
            "wxc": np.ascontiguousarray(Wc[:E]).astype(BF16),
            "wxcm": wxcm,
        }
        m.update(common)
        in_maps.append(m)
    return in_maps


def run_cores(in_maps, trace=False):
    from concourse.bass_utils import run_bass_kernel_spmd
    from concourse.bass_interp import get_hw_module

    if "nc" not in _cache:
        _cache["nc"] = _build()
    nc = _cache["nc"]
    old = nc.m
    nc.m = _cache.setdefault("hwm", get_hw_module(nc.m))
    try:
        res = run_bass_kernel_spmd(nc, in_maps, core_ids=list(range(NCORES)),
                                   trace=trace)
    finally:
        nc.m = old
    return res


def kernel(**inputs):
    in_maps = _prep_inputs(inputs)
    res = run_cores(in_maps)
    logits = np.zeros((B, B, 2), np.float32)
    for core in range(NCORES):
        o = res.results[core]["out"]               # [2, 32*256]
        # pair layout: col = 512*pr + 256*ii + j, local row = 16*ii + pr
        logits[IBLK * core:IBLK * core + IBLK] = \
            o.reshape(2, 16, 2, B).transpose(2, 1, 3, 0).reshape(IBLK, B, 2)
    logits += inputs["b2"].astype(np.float32)[None, None, :]
    pos = logits[np.arange(B), np.arange(B)]
    qi, ri = np.nonzero(~np.eye(B, dtype=bool))
    neg = logits[qi, ri]
    return np.concatenate([pos, neg], axis=0).astype(np.float32)


if __name__ == "__main__":
    _build()
    print("build OK")
